# revision 6
# baseline (speedup 1.0000x reference)
"""GQA attention (B=2,T=2048,E=2048,H=16,Hkv=8,D=128) with RoPE + causal mask,
sharded over 8 NeuronCores: core = (batch b, head-group g) with q-heads
{2g,2g+1,2g+8,2g+9} and kv-heads {2g,2g+1}. Each core computes its 4 heads'
attention for the full sequence plus its partial output projection; the host
sums the 4 partials per batch.

Device layout notes:
 - All matmuls run as float32r (full-rate fp32 on PE for moving dim >= 256).
 - Q/K are produced directly in [d, t] (transposed) layout with even/odd
   RoPE deinterleave folded into host-permuted weight columns; RoPE rotation
   is 4 DVE ops per [128,512] tile using cross-base-partition reads (one
   operand kept in PSUM, which the BIR verifier allows).
 - Scores are computed as S^T [kv,q] so no P transposes are needed; softmax
   denominators come from a ones-column matmul; normalization is applied to
   the [D,q] attention output via a K=1 broadcast matmul + DVE multiply.
"""
import sys
if "/opt/trn_rl_repo" not in sys.path:
    sys.path.insert(0, "/opt/trn_rl_repo")
from contextlib import ExitStack

import numpy as np

import concourse.bass as bass
import concourse.tile as tile
from concourse import bacc, mybir
from concourse.bass_utils import run_bass_kernel_spmd

F32 = mybir.dt.float32
F32R = mybir.dt.float32r
EXP = mybir.ActivationFunctionType.Exp

B, T, E = 2, 2048, 2048
H, H_KV, D = 16, 8, 128
TG = 512                 # q-group / moving-dim tile
NTG = T // TG            # 4
NEC = E // 128           # 16 contraction chunks for projections
SCALE = float(D) ** -0.5

_cache: dict = {}


def _build_program(cfg):
    """cfg: tuple per qg of tuple of (chunk_index, mask_tile_idx or -1)."""
    nmt = max(1, 1 + max((mi for qgc in cfg for _, mi in qgc), default=-1))
    nc = bacc.Bacc("TRN2", target_bir_lowering=False, debug=False, num_devices=8)

    xT = nc.dram_tensor("xT", [E, T], F32R, kind="ExternalInput").ap()
    wqa = nc.dram_tensor("wqa", [E, 4 * D], F32R, kind="ExternalInput").ap()
    wka = nc.dram_tensor("wka", [E, 2 * D], F32R, kind="ExternalInput").ap()
    wvg = nc.dram_tensor("wvg", [E, 2 * D], F32R, kind="ExternalInput").ap()
    wog = nc.dram_tensor("wog", [4 * D, E], F32R, kind="ExternalInput").ap()
    c2d = nc.dram_tensor("c2", [128, T], F32, kind="ExternalInput").ap()
    s2d = nc.dram_tensor("s2", [128, T], F32, kind="ExternalInput").ap()
    onesd = nc.dram_tensor("ones", [128, 128], F32R, kind="ExternalInput").ap()
    mtd = nc.dram_tensor("mt", [nmt, 128, TG], F32R, kind="ExternalInput").ap()
    out = nc.dram_tensor("o", [T, E], F32, kind="ExternalOutput").ap()

    with tile.TileContext(nc) as tc, ExitStack() as ctx:
        cp = ctx.enter_context(tc.tile_pool(name="const", bufs=1))
        big = ctx.enter_context(tc.tile_pool(name="big", bufs=3, space="PSUM"))
        small = ctx.enter_context(tc.tile_pool(name="small", bufs=2, space="PSUM"))
        xp = ctx.enter_context(tc.tile_pool(name="xp", bufs=3))
        csp = ctx.enter_context(tc.tile_pool(name="csp", bufs=2))
        m2p = ctx.enter_context(tc.tile_pool(name="m2p", bufs=2))
        ptp = ctx.enter_context(tc.tile_pool(name="ptp", bufs=3))
        atp = ctx.enter_context(tc.tile_pool(name="atp", bufs=1))
        obp = ctx.enter_context(tc.tile_pool(name="obp", bufs=2))
        rcp = ctx.enter_context(tc.tile_pool(name="rcp", bufs=2))

        # ---- resident constants ----
        wq_t, wk_t, wv_t = [], [], []
        for e in range(NEC):
            wt = cp.tile([128, 4 * D], F32R, tag=f"wq{e}", name=f"wq{e}")
            nc.sync.dma_start(wt[:], wqa[e * 128:(e + 1) * 128, :])
            wq_t.append(wt)
            kt = cp.tile([128, 2 * D], F32R, tag=f"wk{e}", name=f"wk{e}")
            nc.sync.dma_start(kt[:], wka[e * 128:(e + 1) * 128, :])
            wk_t.append(kt)
            vt = cp.tile([128, 2 * D], F32R, tag=f"wv{e}", name=f"wv{e}")
            nc.sync.dma_start(vt[:], wvg[e * 128:(e + 1) * 128, :])
            wv_t.append(vt)
        wo_t = []
        for lh in range(4):
            ot = cp.tile([128, E], F32R, tag=f"wo{lh}", name=f"wo{lh}")
            nc.sync.dma_start(ot[:], wog[lh * 128:(lh + 1) * 128, :])
            wo_t.append(ot)
        ones_col = cp.tile([128, 1], F32R, tag="ones_col", name="ones_col")
        nc.sync.dma_start(ones_col[:], onesd[:, 0:1])
        ones_row = cp.tile([1, 128], F32R, tag="ones_row", name="ones_row")
        nc.sync.dma_start(ones_row[:], onesd[0:1, :])
        mt_t = []
        for i in range(nmt):
            mtt = cp.tile([128, TG], F32R, tag=f"mt{i}", name=f"mt{i}")
            nc.sync.dma_start(mtt[:], mtd[i, :, :])
            mt_t.append(mtt)
        # K^T (rotated) and V persist for the whole sequence
        krot = [cp.tile([128, T], F32R, tag=f"kr{lk}", name=f"kr{lk}") for lk in range(2)]
        vres = [cp.tile([128, 2 * D], F32R, tag=f"v{t}", name=f"v{t}") for t in range(T // 128)]

        for tg in range(NTG):
            tgs = slice(tg * TG, (tg + 1) * TG)
            c2t = csp.tile([128, TG], F32, tag="c2t", name=f"c2t{tg}")
            nc.sync.dma_start(c2t[:], c2d[:, tgs])
            s2t = csp.tile([128, TG], F32, tag="s2t", name=f"s2t{tg}")
            nc.sync.dma_start(s2t[:], s2d[:, tgs])

            # ---- projections: Q^T (4 heads) and K^T (2 kv heads), e-contraction ----
            qp = [big.tile([128, 2 * TG], F32, tag="big", name=f"qp{i}_{tg}") for i in range(2)]
            kp = big.tile([128, 2 * TG], F32, tag="big", name=f"kp_{tg}")
            for e in range(NEC):
                xt = xp.tile([128, TG], F32R, tag="xt", name=f"xt{tg}_{e}")
                nc.sync.dma_start(xt[:], xT[e * 128:(e + 1) * 128, tgs])
                for lh in range(4):
                    nc.tensor.matmul(qp[lh // 2][:, (lh % 2) * TG:(lh % 2 + 1) * TG],
                                     wq_t[e][:, lh * D:(lh + 1) * D], xt[:],
                                     start=(e == 0), stop=(e == NEC - 1))
                for lk in range(2):
                    nc.tensor.matmul(kp[:, lk * TG:(lk + 1) * TG],
                                     wk_t[e][:, lk * D:(lk + 1) * D], xt[:],
                                     start=(e == 0), stop=(e == NEC - 1))

            # ---- RoPE rotation (4 DVE ops per [128,512] tile) ----
            qrot = [atp.tile([128, TG], F32R, tag=f"qr{lh}", name=f"qr{lh}_{tg}")
                    for lh in range(4)]

            def rope(dst, src):
                m2 = m2p.tile([128, TG], F32, tag="m2")
                nc.vector.tensor_mul(m2[:], src, s2t[:])
                nc.vector.tensor_mul(src, src, c2t[:])
                nc.vector.tensor_sub(dst[0:64, :], src[0:64, :], m2[64:128, :])
                nc.vector.tensor_add(dst[64:128, :], src[64:128, :], m2[0:64, :])

            for lh in range(4):
                rope(qrot[lh][:], qp[lh // 2][:, (lh % 2) * TG:(lh % 2 + 1) * TG])
            for lk in range(2):
                rope(krot[lk][:, tgs], kp[:, lk * TG:(lk + 1) * TG])

            # ---- V projection (natural layout), ts pairs share a PSUM bank ----
            for tsp in range(2):
                vp = small.tile([128, 2 * 2 * D], F32, tag="small", name=f"vp{tsp}_{tg}")
                for e in range(NEC):
                    xt2 = xp.tile([128, TG], F32R, tag="xt", name=f"x2{tg}_{tsp}_{e}")
                    nc.sync.dma_start(xt2[:], xT[e * 128:(e + 1) * 128, tgs])
                    for k in range(2):
                        ts = tsp * 2 + k
                        nc.tensor.matmul(vp[:, k * 256:(k + 1) * 256],
                                         xt2[:, ts * 128:(ts + 1) * 128], wv_t[e][:],
                                         start=(e == 0 and k == 0), stop=(e == NEC - 1),
                                         skip_group_check=True)
                for k in range(2):
                    ts = tsp * 2 + k
                    nc.vector.tensor_copy(vres[tg * 4 + ts][:], vp[:, k * 256:(k + 1) * 256])

            # ---- attention for q-group qg=tg ----
            qg = tg
            chunks = cfg[qg]
            nck = len(chunks)
            at_t = []
            for lh in range(4):
                lk = lh % 2
                if nck == 0:
                    at = atp.tile([128, TG], F32R, tag=f"at{lh}", name=f"at{qg}_{lh}")
                    nc.vector.memset(at[:], 0.0)
                    at_t.append(at)
                    continue
                od = small.tile([128, TG], F32, tag="small", name=f"od{qg}_{lh}")
                dd = small.tile([1, TG], F32, tag="small", name=f"dd{qg}_{lh}")
                pairs = [chunks[i:i + 2] for i in range(0, nck, 2)]

                def emit_av(pt, pair, base_idx):
                    for k, (c, mi) in enumerate(pair):
                        first = (base_idx + k == 0)
                        last = (base_idx + k == nck - 1)
                        nc.tensor.matmul(od[:],
                                         vres[c][:, lk * D:(lk + 1) * D],
                                         pt[:, k * TG:(k + 1) * TG],
                                         start=first, stop=last, skip_group_check=True)
                        nc.tensor.matmul(dd[:], ones_col[:],
                                         pt[:, k * TG:(k + 1) * TG],
                                         start=first, stop=last, skip_group_check=True)

                def emit_s_exp(p, pair):
                    sp = big.tile([128, 2 * TG], F32, tag="big", name=f"sp{qg}_{lh}_{p}")
                    for k, (c, mi) in enumerate(pair):
                        nc.tensor.matmul(sp[:, k * TG:(k + 1) * TG],
                                         krot[lk][:, c * 128:(c + 1) * 128],
                                         qrot[lh][:], start=True, stop=True)
                    pt = ptp.tile([128, 2 * TG], F32R, tag="pt", name=f"pt{qg}_{lh}_{p}")
                    w = len(pair) * TG
                    nc.scalar.activation(pt[:, 0:w], sp[:, 0:w], EXP, scale=SCALE)
                    for k, (c, mi) in enumerate(pair):
                        if mi >= 0:
                            nc.vector.tensor_mul(pt[:, k * TG:(k + 1) * TG],
                                                 pt[:, k * TG:(k + 1) * TG], mt_t[mi][:])
                    return (pt, pair, 2 * p)

                # software pipeline: keep two S pairs in flight ahead of AV
                pending = []
                for p, pair in enumerate(pairs):
                    pending.append(emit_s_exp(p, pair))
                    if len(pending) > 2:
                        emit_av(*pending.pop(0))
                for item in pending:
                    emit_av(*item)

                # normalization: attnT = od * bcast(1/denom)
                den = rcp.tile([1, TG], F32R, tag="den", name=f"den{qg}_{lh}")
                nc.scalar.activation(den[:], dd[:], mybir.ActivationFunctionType.Copy,
                                     bias=1e-30)
                bt = big.tile([128, 2 * TG], F32, tag="big", name=f"bt{qg}_{lh}")
                nc.tensor.matmul(bt[:, 0:TG], ones_row[:], den[:], start=True, stop=True)
                rec = rcp.tile([128, TG], F32, tag="rec", name=f"rec{qg}_{lh}")
                nc.vector.reciprocal(rec[:], bt[:, 0:TG])
                at = atp.tile([128, TG], F32R, tag=f"at{lh}", name=f"at{qg}_{lh}")
                with nc.allow_low_precision(reason="fp32r attn output for wo matmul"):
                    nc.vector.tensor_mul(at[:], od[:], rec[:])
                at_t.append(at)

            # ---- output projection for this q-group ----
            for ts in range(4):
                for eb in range(NTG):
                    wps = small.tile([128, TG], F32, tag="small", name=f"w{qg}_{ts}_{eb}")
                    for lh in range(4):
                        nc.tensor.matmul(wps[:], at_t[lh][:, ts * 128:(ts + 1) * 128],
                                         wo_t[lh][:, eb * TG:(eb + 1) * TG],
                                         start=(lh == 0), stop=(lh == 3))
                    ob = obp.tile([128, TG], F32, tag="ob", name=f"ob{qg}_{ts}_{eb}")
                    if (ts + eb) % 2 == 0:
                        nc.scalar.copy(ob[:], wps[:])
                    else:
                        nc.vector.tensor_copy(ob[:], wps[:])
                    nc.sync.dma_start(
                        out[qg * TG + ts * 128:qg * TG + (ts + 1) * 128,
                            eb * TG:(eb + 1) * TG], ob[:])

    nc.compile()
    return nc


def _host_prep(x, mask, wq, wk, wv, wo):
    m2dm = np.asarray(mask).reshape(T, T) != 0
    cfg = []
    mask_tiles = []
    mask_key = {}
    for qg in range(NTG):
        qs = slice(qg * TG, (qg + 1) * TG)
        lst = []
        for c in range(T // 128):
            sub = m2dm[qs, c * 128:(c + 1) * 128]
            if not sub.any():
                continue
            if sub.all():
                lst.append((c, -1))
            else:
                tilea = np.ascontiguousarray(sub.T).astype(np.float32)
                key = tilea.tobytes()
                if key not in mask_key:
                    mask_key[key] = len(mask_tiles)
                    mask_tiles.append(tilea)
                lst.append((c, mask_key[key]))
        cfg.append(tuple(lst))
    cfg = tuple(cfg)
    if mask_tiles:
        mt = np.stack(mask_tiles, 0)
    else:
        mt = np.zeros((1, 128, TG), np.float32)

    inv = 1.0 / (10000.0 ** (np.arange(64, dtype=np.float64) / 64))
    ang = np.arange(T, dtype=np.float64)[:, None] * inv[None, :]
    c64 = np.cos(ang).astype(np.float32).T
    s64 = np.sin(ang).astype(np.float32).T
    c2 = np.ascontiguousarray(np.concatenate([c64, c64], 0))
    s2 = np.ascontiguousarray(np.concatenate([s64, s64], 0))
    ones = np.ones((128, 128), np.float32)

    in_maps = []
    for b in range(B):
        xTb = np.ascontiguousarray(x[b].T)
        for g in range(4):
            heads = [2 * g, 2 * g + 1, 2 * g + 8, 2 * g + 9]
            kvh = [2 * g, 2 * g + 1]
            wq_a = np.empty((E, 4 * D), np.float32)
            for lh, h in enumerate(heads):
                wq_a[:, lh * D:lh * D + 64] = wq[:, h * D:(h + 1) * D:2]
                wq_a[:, lh * D + 64:(lh + 1) * D] = wq[:, h * D + 1:(h + 1) * D:2]
            wk_a = np.empty((E, 2 * D), np.float32)
            for lk, h in enumerate(kvh):
                wk_a[:, lk * D:lk * D + 64] = wk[:, h * D:(h + 1) * D:2]
                wk_a[:, lk * D + 64:(lk + 1) * D] = wk[:, h * D + 1:(h + 1) * D:2]
            wv_g = np.ascontiguousarray(wv[:, kvh[0] * D:(kvh[0] + 2) * D])
            wo_g = np.ascontiguousarray(
                np.concatenate([wo[h * D:(h + 1) * D] for h in heads], 0))
            in_maps.append({
                "xT": xTb, "wqa": wq_a, "wka": wk_a, "wvg": wv_g, "wog": wo_g,
                "c2": c2, "s2": s2, "ones": ones, "mt": mt,
            })
    return cfg, in_maps


def kernel(x, mask, wq, wk, wv, wo, _profile=None):
    x = np.ascontiguousarray(np.asarray(x, dtype=np.float32))
    wq = np.asarray(wq, dtype=np.float32)
    wk = np.asarray(wk, dtype=np.float32)
    wv = np.asarray(wv, dtype=np.float32)
    wo = np.asarray(wo, dtype=np.float32)
    cfg, in_maps = _host_prep(x, mask, wq, wk, wv, wo)
    if cfg not in _cache:
        _cache[cfg] = _build_program(cfg)
    nc = _cache[cfg]
    kwargs = dict(_profile) if _profile else {}
    res = run_bass_kernel_spmd(nc, in_maps, core_ids=list(range(8)), **kwargs)
    if _profile is not None and isinstance(_profile, dict):
        _profile["result"] = res
    outp = np.zeros((B, T, E), np.float32)
    for b in range(B):
        for g in range(4):
            outp[b] += res.results[b * 4 + g]["o"]
    return outp


# revision 7
# speedup vs baseline: 1.1054x; 1.1054x over previous
"""GQA attention (B=2,T=2048,E=2048,H=16,Hkv=8,D=128) with RoPE + causal mask,
sharded over 8 NeuronCores: core = (batch b, head-group g) with q-heads
{2g,2g+1,2g+8,2g+9} and kv-heads {2g,2g+1}. Each core computes its 4 heads'
attention for the full sequence plus its partial output projection; the host
sums the 4 partials per batch.

Device layout notes:
 - All matmuls run as float32r (full-rate fp32 on PE for moving dim >= 256).
 - Q/K are produced directly in [d, t] (transposed) layout with even/odd
   RoPE deinterleave folded into host-permuted weight columns; RoPE rotation
   is 4 DVE ops per [128,512] tile using cross-base-partition reads (one
   operand kept in PSUM, which the BIR verifier allows).
 - Scores are computed as S^T [kv,q] so no P transposes are needed; softmax
   denominators come from a ones-column matmul; normalization is applied to
   the [D,q] attention output via a K=1 broadcast matmul + DVE multiply.
"""
import sys
if "/opt/trn_rl_repo" not in sys.path:
    sys.path.insert(0, "/opt/trn_rl_repo")
from contextlib import ExitStack

import numpy as np

import concourse.bass as bass
import concourse.tile as tile
from concourse import bacc, mybir
from concourse.bass_utils import run_bass_kernel_spmd

F32 = mybir.dt.float32
F32R = mybir.dt.float32r
EXP = mybir.ActivationFunctionType.Exp

B, T, E = 2, 2048, 2048
H, H_KV, D = 16, 8, 128
TG = 512                 # q-group / moving-dim tile
NTG = T // TG            # 4
NEC = E // 128           # 16 contraction chunks for projections
SCALE = float(D) ** -0.5

_cache: dict = {}


def _build_program(cfg):
    """cfg: tuple per qg of tuple of (chunk_index, mask_tile_idx or -1)."""
    nmt = max(1, 1 + max((mi for qgc in cfg for _, mi in qgc), default=-1))
    nc = bacc.Bacc("TRN2", target_bir_lowering=False, debug=False, num_devices=8)

    xT = nc.dram_tensor("xT", [E, T], F32R, kind="ExternalInput").ap()
    wqa = nc.dram_tensor("wqa", [E, 4 * D], F32R, kind="ExternalInput").ap()
    wka = nc.dram_tensor("wka", [E, 2 * D], F32R, kind="ExternalInput").ap()
    wvg = nc.dram_tensor("wvg", [E, 2 * D], F32R, kind="ExternalInput").ap()
    wog = nc.dram_tensor("wog", [4 * D, E], F32R, kind="ExternalInput").ap()
    c2d = nc.dram_tensor("c2", [128, T], F32, kind="ExternalInput").ap()
    s2d = nc.dram_tensor("s2", [128, T], F32, kind="ExternalInput").ap()
    onesd = nc.dram_tensor("ones", [128, 128], F32R, kind="ExternalInput").ap()
    mtd = nc.dram_tensor("mt", [nmt, 128, TG], F32R, kind="ExternalInput").ap()
    out = nc.dram_tensor("o", [T, E], F32, kind="ExternalOutput").ap()

    with tile.TileContext(nc) as tc, ExitStack() as ctx:
        cp = ctx.enter_context(tc.tile_pool(name="const", bufs=1))
        big = ctx.enter_context(tc.tile_pool(name="big", bufs=3, space="PSUM"))
        small = ctx.enter_context(tc.tile_pool(name="small", bufs=2, space="PSUM"))
        xp = ctx.enter_context(tc.tile_pool(name="xp", bufs=3))
        csp = ctx.enter_context(tc.tile_pool(name="csp", bufs=2))
        m2p = ctx.enter_context(tc.tile_pool(name="m2p", bufs=2))
        ptp = ctx.enter_context(tc.tile_pool(name="ptp", bufs=3))
        atp = ctx.enter_context(tc.tile_pool(name="atp", bufs=1))
        obp = ctx.enter_context(tc.tile_pool(name="obp", bufs=2))
        rcp = ctx.enter_context(tc.tile_pool(name="rcp", bufs=2))

        # ---- resident constants ----
        wq_t, wk_t, wv_t = [], [], []
        for e in range(NEC):
            wt = cp.tile([128, 4 * D], F32R, tag=f"wq{e}", name=f"wq{e}")
            nc.sync.dma_start(wt[:], wqa[e * 128:(e + 1) * 128, :])
            wq_t.append(wt)
            kt = cp.tile([128, 2 * D], F32R, tag=f"wk{e}", name=f"wk{e}")
            nc.sync.dma_start(kt[:], wka[e * 128:(e + 1) * 128, :])
            wk_t.append(kt)
            vt = cp.tile([128, 2 * D], F32R, tag=f"wv{e}", name=f"wv{e}")
            nc.sync.dma_start(vt[:], wvg[e * 128:(e + 1) * 128, :])
            wv_t.append(vt)
        wo_t = []
        for lh in range(4):
            ot = cp.tile([128, E], F32R, tag=f"wo{lh}", name=f"wo{lh}")
            nc.sync.dma_start(ot[:], wog[lh * 128:(lh + 1) * 128, :])
            wo_t.append(ot)
        ones_col = cp.tile([128, 1], F32R, tag="ones_col", name="ones_col")
        nc.sync.dma_start(ones_col[:], onesd[:, 0:1])
        ones_row = cp.tile([1, 128], F32R, tag="ones_row", name="ones_row")
        nc.sync.dma_start(ones_row[:], onesd[0:1, :])
        mt_t = []
        for i in range(nmt):
            mtt = cp.tile([128, TG], F32R, tag=f"mt{i}", name=f"mt{i}")
            nc.sync.dma_start(mtt[:], mtd[i, :, :])
            mt_t.append(mtt)
        # K^T (rotated) and V persist for the whole sequence
        krot = [cp.tile([128, T], F32R, tag=f"kr{lk}", name=f"kr{lk}") for lk in range(2)]
        vres = [cp.tile([128, 2 * D], F32R, tag=f"v{t}", name=f"v{t}") for t in range(T // 128)]

        for tg in range(NTG):
            tgs = slice(tg * TG, (tg + 1) * TG)
            c2t = csp.tile([128, TG], F32, tag="c2t", name=f"c2t{tg}")
            nc.sync.dma_start(c2t[:], c2d[:, tgs])
            s2t = csp.tile([128, TG], F32, tag="s2t", name=f"s2t{tg}")
            nc.sync.dma_start(s2t[:], s2d[:, tgs])

            # ---- projections: Q^T (4 heads) and K^T (2 kv heads), e-contraction ----
            qp = [big.tile([128, 2 * TG], F32, tag="big", name=f"qp{i}_{tg}") for i in range(2)]
            kp = big.tile([128, 2 * TG], F32, tag="big", name=f"kp_{tg}")
            for e in range(NEC):
                xt = xp.tile([128, TG], F32R, tag="xt", name=f"xt{tg}_{e}")
                nc.sync.dma_start(xt[:], xT[e * 128:(e + 1) * 128, tgs])
                for lh in range(4):
                    nc.tensor.matmul(qp[lh // 2][:, (lh % 2) * TG:(lh % 2 + 1) * TG],
                                     wq_t[e][:, lh * D:(lh + 1) * D], xt[:],
                                     start=(e == 0), stop=(e == NEC - 1))
                for lk in range(2):
                    nc.tensor.matmul(kp[:, lk * TG:(lk + 1) * TG],
                                     wk_t[e][:, lk * D:(lk + 1) * D], xt[:],
                                     start=(e == 0), stop=(e == NEC - 1))

            # ---- RoPE rotation (4 DVE ops per [128,512] tile) ----
            qrot = [atp.tile([128, TG], F32R, tag=f"qr{lh}", name=f"qr{lh}_{tg}")
                    for lh in range(4)]

            def rope(dst, src):
                m2 = m2p.tile([128, TG], F32, tag="m2")
                nc.vector.tensor_mul(m2[:], src, s2t[:])
                nc.vector.tensor_mul(src, src, c2t[:])
                nc.vector.tensor_sub(dst[0:64, :], src[0:64, :], m2[64:128, :])
                nc.vector.tensor_add(dst[64:128, :], src[64:128, :], m2[0:64, :])

            for lh in range(4):
                rope(qrot[lh][:], qp[lh // 2][:, (lh % 2) * TG:(lh % 2 + 1) * TG])
            for lk in range(2):
                rope(krot[lk][:, tgs], kp[:, lk * TG:(lk + 1) * TG])

            # ---- V projection (natural layout), ts pairs share a PSUM bank ----
            for tsp in range(2):
                vp = small.tile([128, 2 * 2 * D], F32, tag="small", name=f"vp{tsp}_{tg}")
                for e in range(NEC):
                    xt2 = xp.tile([128, TG], F32R, tag="xt", name=f"x2{tg}_{tsp}_{e}")
                    nc.sync.dma_start(xt2[:], xT[e * 128:(e + 1) * 128, tgs])
                    for k in range(2):
                        ts = tsp * 2 + k
                        nc.tensor.matmul(vp[:, k * 256:(k + 1) * 256],
                                         xt2[:, ts * 128:(ts + 1) * 128], wv_t[e][:],
                                         start=(e == 0 and k == 0), stop=(e == NEC - 1),
                                         skip_group_check=True)
                for k in range(2):
                    ts = tsp * 2 + k
                    nc.vector.tensor_copy(vres[tg * 4 + ts][:], vp[:, k * 256:(k + 1) * 256])

            # ---- attention for q-group qg=tg ----
            qg = tg
            chunks = cfg[qg]
            nck = len(chunks)
            at_t = []
            for lh in range(4):
                lk = lh % 2
                if nck == 0:
                    at = atp.tile([128, TG], F32R, tag=f"at{lh}", name=f"at{qg}_{lh}")
                    nc.vector.memset(at[:], 0.0)
                    at_t.append(at)
                    continue
                od = small.tile([128, TG], F32, tag="small", name=f"od{qg}_{lh}")
                dd = small.tile([1, TG], F32, tag="small", name=f"dd{qg}_{lh}")
                pairs = [chunks[i:i + 2] for i in range(0, nck, 2)]

                def emit_av(pt, pair, base_idx):
                    for k, (c, mi) in enumerate(pair):
                        first = (base_idx + k == 0)
                        last = (base_idx + k == nck - 1)
                        nc.tensor.matmul(od[:],
                                         vres[c][:, lk * D:(lk + 1) * D],
                                         pt[:, k * TG:(k + 1) * TG],
                                         start=first, stop=last, skip_group_check=True)
                        nc.tensor.matmul(dd[:], ones_col[:],
                                         pt[:, k * TG:(k + 1) * TG],
                                         start=first, stop=last, skip_group_check=True)

                def emit_s_exp(p, pair):
                    sp = big.tile([128, 2 * TG], F32, tag="big", name=f"sp{qg}_{lh}_{p}")
                    for k, (c, mi) in enumerate(pair):
                        nc.tensor.matmul(sp[:, k * TG:(k + 1) * TG],
                                         krot[lk][:, c * 128:(c + 1) * 128],
                                         qrot[lh][:], start=True, stop=True)
                    pt = ptp.tile([128, 2 * TG], F32R, tag="pt", name=f"pt{qg}_{lh}_{p}")
                    w = len(pair) * TG
                    nc.scalar.activation(pt[:, 0:w], sp[:, 0:w], EXP, scale=SCALE)
                    for k, (c, mi) in enumerate(pair):
                        if mi >= 0:
                            nc.vector.tensor_mul(pt[:, k * TG:(k + 1) * TG],
                                                 pt[:, k * TG:(k + 1) * TG], mt_t[mi][:])
                    return (pt, pair, 2 * p)

                # software pipeline: keep two S pairs in flight ahead of AV
                pending = []
                for p, pair in enumerate(pairs):
                    pending.append(emit_s_exp(p, pair))
                    if len(pending) > 2:
                        emit_av(*pending.pop(0))
                for item in pending:
                    emit_av(*item)

                # normalization: attnT = od * bcast(1/denom)
                den = rcp.tile([1, TG], F32R, tag="den", name=f"den{qg}_{lh}")
                nc.scalar.activation(den[:], dd[:], mybir.ActivationFunctionType.Copy,
                                     bias=1e-30)
                bt = small.tile([128, TG], F32, tag="small", name=f"bt{qg}_{lh}")
                nc.tensor.matmul(bt[:], ones_row[:], den[:], start=True, stop=True)
                rec = rcp.tile([128, TG], F32, tag="rec", name=f"rec{qg}_{lh}")
                nc.vector.reciprocal_approx_fast(rec[:], bt[:])
                at = atp.tile([128, TG], F32R, tag=f"at{lh}", name=f"at{qg}_{lh}")
                with nc.allow_low_precision(reason="fp32r attn output for wo matmul"):
                    nc.vector.tensor_mul(at[:], od[:], rec[:])
                at_t.append(at)

            # ---- output projection for this q-group ----
            for ts in range(4):
                for eb in range(NTG):
                    wps = small.tile([128, TG], F32, tag="small", name=f"w{qg}_{ts}_{eb}")
                    for lh in range(4):
                        nc.tensor.matmul(wps[:], at_t[lh][:, ts * 128:(ts + 1) * 128],
                                         wo_t[lh][:, eb * TG:(eb + 1) * TG],
                                         start=(lh == 0), stop=(lh == 3))
                    ob = obp.tile([128, TG], F32, tag="ob", name=f"ob{qg}_{ts}_{eb}")
                    if (ts + eb) % 2 == 0:
                        nc.scalar.copy(ob[:], wps[:])
                    else:
                        nc.vector.tensor_copy(ob[:], wps[:])
                    nc.sync.dma_start(
                        out[qg * TG + ts * 128:qg * TG + (ts + 1) * 128,
                            eb * TG:(eb + 1) * TG], ob[:])

    nc.compile()
    return nc


def _host_prep(x, mask, wq, wk, wv, wo):
    m2dm = np.asarray(mask).reshape(T, T) != 0
    cfg = []
    mask_tiles = []
    mask_key = {}
    for qg in range(NTG):
        qs = slice(qg * TG, (qg + 1) * TG)
        lst = []
        for c in range(T // 128):
            sub = m2dm[qs, c * 128:(c + 1) * 128]
            if not sub.any():
                continue
            if sub.all():
                lst.append((c, -1))
            else:
                tilea = np.ascontiguousarray(sub.T).astype(np.float32)
                key = tilea.tobytes()
                if key not in mask_key:
                    mask_key[key] = len(mask_tiles)
                    mask_tiles.append(tilea)
                lst.append((c, mask_key[key]))
        cfg.append(tuple(lst))
    cfg = tuple(cfg)
    if mask_tiles:
        mt = np.stack(mask_tiles, 0)
    else:
        mt = np.zeros((1, 128, TG), np.float32)

    inv = 1.0 / (10000.0 ** (np.arange(64, dtype=np.float64) / 64))
    ang = np.arange(T, dtype=np.float64)[:, None] * inv[None, :]
    c64 = np.cos(ang).astype(np.float32).T
    s64 = np.sin(ang).astype(np.float32).T
    c2 = np.ascontiguousarray(np.concatenate([c64, c64], 0))
    s2 = np.ascontiguousarray(np.concatenate([s64, s64], 0))
    ones = np.ones((128, 128), np.float32)

    in_maps = []
    for b in range(B):
        xTb = np.ascontiguousarray(x[b].T)
        for g in range(4):
            heads = [2 * g, 2 * g + 1, 2 * g + 8, 2 * g + 9]
            kvh = [2 * g, 2 * g + 1]
            wq_a = np.empty((E, 4 * D), np.float32)
            for lh, h in enumerate(heads):
                wq_a[:, lh * D:lh * D + 64] = wq[:, h * D:(h + 1) * D:2]
                wq_a[:, lh * D + 64:(lh + 1) * D] = wq[:, h * D + 1:(h + 1) * D:2]
            wk_a = np.empty((E, 2 * D), np.float32)
            for lk, h in enumerate(kvh):
                wk_a[:, lk * D:lk * D + 64] = wk[:, h * D:(h + 1) * D:2]
                wk_a[:, lk * D + 64:(lk + 1) * D] = wk[:, h * D + 1:(h + 1) * D:2]
            wv_g = np.ascontiguousarray(wv[:, kvh[0] * D:(kvh[0] + 2) * D])
            wo_g = np.ascontiguousarray(
                np.concatenate([wo[h * D:(h + 1) * D] for h in heads], 0))
            in_maps.append({
                "xT": xTb, "wqa": wq_a, "wka": wk_a, "wvg": wv_g, "wog": wo_g,
                "c2": c2, "s2": s2, "ones": ones, "mt": mt,
            })
    return cfg, in_maps


def kernel(x, mask, wq, wk, wv, wo, _profile=None):
    x = np.ascontiguousarray(np.asarray(x, dtype=np.float32))
    wq = np.asarray(wq, dtype=np.float32)
    wk = np.asarray(wk, dtype=np.float32)
    wv = np.asarray(wv, dtype=np.float32)
    wo = np.asarray(wo, dtype=np.float32)
    cfg, in_maps = _host_prep(x, mask, wq, wk, wv, wo)
    if cfg not in _cache:
        _cache[cfg] = _build_program(cfg)
    nc = _cache[cfg]
    kwargs = dict(_profile) if _profile else {}
    res = run_bass_kernel_spmd(nc, in_maps, core_ids=list(range(8)), **kwargs)
    if _profile is not None and isinstance(_profile, dict):
        _profile["result"] = res
    outp = np.zeros((B, T, E), np.float32)
    for b in range(B):
        for g in range(4):
            outp[b] += res.results[b * 4 + g]["o"]
    return outp


# revision 15
# speedup vs baseline: 1.6925x; 1.5311x over previous
"""GQA attention (B=2,T=2048,E=2048,H=16,Hkv=8,D=128) with RoPE + causal mask,
sharded over 8 NeuronCores: core = (batch b, head-group g) with q-heads
{2g,2g+1,2g+8,2g+9} and kv-heads {2g,2g+1}. Each core computes its 4 heads'
attention for the full sequence plus its partial output projection; the host
sums the 4 partials per batch.

Device layout notes:
 - All matmuls run as float32r (full-rate fp32 on PE for moving dim >= 256).
 - Q/K are produced directly in [d, t] (transposed) layout with even/odd
   RoPE deinterleave folded into host-permuted weight columns; RoPE rotation
   is 4 DVE ops per [128,512] tile using cross-base-partition reads (one
   operand kept in PSUM, which the BIR verifier allows).
 - Scores are computed as S^T [kv,q] so no P transposes are needed; softmax
   denominators come from a ones-column matmul; normalization is applied to
   the [D,q] attention output via a K=1 broadcast matmul + DVE multiply.
"""
import sys
if "/opt/trn_rl_repo" not in sys.path:
    sys.path.insert(0, "/opt/trn_rl_repo")
from contextlib import ExitStack

import numpy as np

import concourse.bass as bass
import concourse.tile as tile
from concourse import bacc, mybir
from concourse.bass_utils import run_bass_kernel_spmd

F32 = mybir.dt.float32
F32R = mybir.dt.float32r
EXP = mybir.ActivationFunctionType.Exp

B, T, E = 2, 2048, 2048
H, H_KV, D = 16, 8, 128
TG = 512                 # q-group / moving-dim tile
NTG = T // TG            # 4
NEC = E // 128           # 16 contraction chunks for projections
SCALE = float(D) ** -0.5

_cache: dict = {}


def _build_program(cfg):
    """cfg: tuple per qg of tuple of (chunk_index, mask_tile_idx or -1)."""
    nmt = max(1, 1 + max((mi for qgc in cfg for _, mi in qgc), default=-1))
    nc = bacc.Bacc("TRN2", target_bir_lowering=False, debug=False, num_devices=8)

    xT = nc.dram_tensor("xT", [E, T], F32R, kind="ExternalInput").ap()
    wqa = nc.dram_tensor("wqa", [E, 4 * D], F32R, kind="ExternalInput").ap()
    wka = nc.dram_tensor("wka", [E, 2 * D], F32R, kind="ExternalInput").ap()
    wvg = nc.dram_tensor("wvg", [E, 2 * D], F32R, kind="ExternalInput").ap()
    wog = nc.dram_tensor("wog", [4 * D, E], F32R, kind="ExternalInput").ap()
    c2d = nc.dram_tensor("c2", [128, T], F32, kind="ExternalInput").ap()
    s2d = nc.dram_tensor("s2", [128, T], F32, kind="ExternalInput").ap()
    onesd = nc.dram_tensor("ones", [128, 128], F32R, kind="ExternalInput").ap()
    mtd = nc.dram_tensor("mt", [nmt, 128, TG], F32R, kind="ExternalInput").ap()
    out = nc.dram_tensor("o", [T, E], F32, kind="ExternalOutput").ap()

    with tile.TileContext(nc) as tc, ExitStack() as ctx:
        cp = ctx.enter_context(tc.tile_pool(name="const", bufs=1))
        pairp = ctx.enter_context(tc.tile_pool(name="pairp", bufs=2, space="PSUM"))
        singp = ctx.enter_context(tc.tile_pool(name="singp", bufs=4, space="PSUM"))
        xp = ctx.enter_context(tc.tile_pool(name="xp", bufs=4))
        csp = ctx.enter_context(tc.tile_pool(name="csp", bufs=2))
        m2p = ctx.enter_context(tc.tile_pool(name="m2p", bufs=2))
        ptp = ctx.enter_context(tc.tile_pool(name="ptp", bufs=4))
        atp = ctx.enter_context(tc.tile_pool(name="atp", bufs=1))
        obp = ctx.enter_context(tc.tile_pool(name="obp", bufs=2))
        rcp = ctx.enter_context(tc.tile_pool(name="rcp", bufs=2))

        # ---- resident constants (heavy DMAs deferred for fast rampup) ----
        wq_t, wk_t, wv_t = [], [], []
        for e in range(NEC):
            wq_t.append(cp.tile([128, 4 * D], F32R, tag=f"wq{e}", name=f"wq{e}"))
            wk_t.append(cp.tile([128, 2 * D], F32R, tag=f"wk{e}", name=f"wk{e}"))
            wv_t.append(cp.tile([128, 2 * D], F32R, tag=f"wv{e}", name=f"wv{e}"))
        wo_t = [cp.tile([128, E], F32R, tag=f"wo{lh}", name=f"wo{lh}") for lh in range(4)]
        ones_col = cp.tile([128, 1], F32R, tag="ones_col", name="ones_col")
        nc.sync.dma_start(ones_col[:], onesd[:, 0:1])
        ones_row = cp.tile([1, 128], F32R, tag="ones_row", name="ones_row")
        nc.sync.dma_start(ones_row[:], onesd[0:1, :])
        mt_t = [cp.tile([128, TG], F32R, tag=f"mt{i}", name=f"mt{i}") for i in range(nmt)]
        # K^T (rotated) and V persist for the whole sequence
        krot = [cp.tile([128, T], F32R, tag=f"kr{lk}", name=f"kr{lk}") for lk in range(2)]
        vres = [cp.tile([128, 2 * D], F32R, tag=f"v{t}", name=f"v{t}") for t in range(T // 128)]

        def emit_proj(tg):
            """Single-pass projections for t-group tg: Q^T(4 heads), K^T(2),
            V(4 t-subtiles as 2 bank-paired accumulators)."""
            tgs = slice(tg * TG, (tg + 1) * TG)
            c2t = csp.tile([128, TG], F32, tag="c2t", name=f"c2t{tg}")
            nc.sync.dma_start(c2t[:], c2d[:, tgs])
            s2t = csp.tile([128, TG], F32, tag="s2t", name=f"s2t{tg}")
            nc.sync.dma_start(s2t[:], s2d[:, tgs])

            qp2 = [pairp.tile([128, 2 * TG], F32, tag="pp", name=f"qp{i}_{tg}") for i in range(2)]
            qp = [qp2[lh // 2][:, (lh % 2) * TG:(lh % 2 + 1) * TG] for lh in range(4)]
            kp = [singp.tile([128, TG], F32, tag="sg", name=f"kp{lk}_{tg}") for lk in range(2)]
            vp = [singp.tile([128, TG], F32, tag="sg", name=f"vp{tsp}_{tg}") for tsp in range(2)]
            for e in range(NEC):
                if tg == 0:
                    nc.sync.dma_start(wq_t[e][:], wqa[e * 128:(e + 1) * 128, :])
                    nc.sync.dma_start(wk_t[e][:], wka[e * 128:(e + 1) * 128, :])
                    nc.sync.dma_start(wv_t[e][:], wvg[e * 128:(e + 1) * 128, :])
                xt = xp.tile([128, TG], F32R, tag="xt", name=f"xt{tg}_{e}")
                nc.sync.dma_start(xt[:], xT[e * 128:(e + 1) * 128, tgs])
                for lh in range(4):
                    nc.tensor.matmul(qp[lh], wq_t[e][:, lh * D:(lh + 1) * D], xt[:],
                                     start=(e == 0), stop=(e == NEC - 1))
                for lk in range(2):
                    nc.tensor.matmul(kp[lk][:], wk_t[e][:, lk * D:(lk + 1) * D], xt[:],
                                     start=(e == 0), stop=(e == NEC - 1))
                for tsp in range(2):
                    for k in range(2):
                        ts = tsp * 2 + k
                        nc.tensor.matmul(vp[tsp][:, k * 256:(k + 1) * 256],
                                         xt[:, ts * 128:(ts + 1) * 128], wv_t[e][:],
                                         start=(e == 0 and k == 0), stop=(e == NEC - 1),
                                         skip_group_check=True)

            # RoPE rotation (4 DVE ops per [128,512] tile, in-place in PSUM)
            qrot = [atp.tile([128, TG], F32R, tag=f"qr{lh}", name=f"qr{lh}_{tg}")
                    for lh in range(4)]

            def rope(dst, src):
                m2 = m2p.tile([128, TG], F32, tag="m2")
                nc.vector.tensor_mul(m2[:], src, s2t[:])
                nc.vector.tensor_mul(src, src, c2t[:])
                nc.vector.tensor_sub(dst[0:64, :], src[0:64, :], m2[64:128, :])
                nc.vector.tensor_add(dst[64:128, :], src[64:128, :], m2[0:64, :])

            for tsp in range(2):
                for k in range(2):
                    ts = tsp * 2 + k
                    nc.scalar.copy(vres[tg * 4 + ts][:], vp[tsp][:, k * 256:(k + 1) * 256])
            rope(krot[0][:, tgs], kp[0][:])
            rope(qrot[0][:], qp[0])
            rope(krot[1][:, tgs], kp[1][:])
            for lh in range(1, 4):
                rope(qrot[lh][:], qp[lh])
            return qrot

        def emit_attention(qg, qrot):
            """Attention for q-group qg. Returns the 4 normalized attnT tiles."""
            chunks = cfg[qg]
            nck = len(chunks)
            at_t = [atp.tile([128, TG], F32R, tag=f"at{lh}", name=f"at{qg}_{lh}")
                    for lh in range(4)]
            norm_pending = []

            def finish_normalize():
                if not norm_pending:
                    return
                lh, od, dd, den = norm_pending.pop()
                bt = singp.tile([128, TG], F32, tag="sg", name=f"bt{qg}_{lh}")
                nc.tensor.matmul(bt[:], ones_row[:], den[:], start=True, stop=True)
                rec = rcp.tile([128, TG], F32, tag="rec", name=f"rec{qg}_{lh}")
                nc.vector.reciprocal_approx_fast(rec[:], bt[:])
                with nc.allow_low_precision(reason="fp32r attn output for wo matmul"):
                    nc.vector.tensor_mul(at_t[lh][:], od[:], rec[:])

            if nck == 0:
                for lh in range(4):
                    nc.vector.memset(at_t[lh][:], 0.0)
                return at_t

            pairs = [chunks[i:i + 2] for i in range(0, nck, 2)]
            for lh in range(4):
                lk = lh % 2
                od = singp.tile([128, TG], F32, tag="sg", name=f"od{qg}_{lh}")
                dd = singp.tile([1, TG], F32, tag="sg", name=f"dd{qg}_{lh}")
                pts = []

                def emit_av_pair(pi):
                    pt, pair = pts[pi]
                    for k, (c, mi) in enumerate(pair):
                        idx = 2 * pi + k
                        nc.tensor.matmul(od[:], vres[c][:, lk * D:(lk + 1) * D],
                                         pt[:, k * TG:(k + 1) * TG],
                                         start=(idx == 0), stop=(idx == nck - 1),
                                         skip_group_check=True)
                        nc.tensor.matmul(dd[:], ones_col[:],
                                         pt[:, k * TG:(k + 1) * TG],
                                         start=(idx == 0), stop=(idx == nck - 1),
                                         skip_group_check=True)

                pending = []
                for pi, pair in enumerate(pairs):
                    sp = pairp.tile([128, 2 * TG], F32, tag="pp", name=f"sp{qg}_{lh}_{pi}")
                    for k, (c, mi) in enumerate(pair):
                        nc.tensor.matmul(sp[:, k * TG:(k + 1) * TG],
                                         krot[lk][:, c * 128:(c + 1) * 128],
                                         qrot[lh][:], start=True, stop=True)
                    pt = ptp.tile([128, 2 * TG], F32R, tag="pt", name=f"pt{qg}_{lh}_{pi}")
                    w = len(pair) * TG
                    nc.scalar.activation(pt[:, 0:w], sp[:, 0:w], EXP, scale=SCALE)
                    for k, (c, mi) in enumerate(pair):
                        if mi >= 0:
                            nc.vector.tensor_mul(pt[:, k * TG:(k + 1) * TG],
                                                 pt[:, k * TG:(k + 1) * TG], mt_t[mi][:])
                    pts.append((pt, pair))
                    pending.append(pi)
                    if pi == 1:
                        finish_normalize()   # previous head, hidden under refill
                    if len(pending) > 1:
                        emit_av_pair(pending.pop(0))
                if len(pairs) < 2:
                    finish_normalize()
                for pi in pending:
                    emit_av_pair(pi)
                den = rcp.tile([1, TG], F32R, tag="den", name=f"den{qg}_{lh}")
                nc.scalar.activation(den[:], dd[:], mybir.ActivationFunctionType.Copy,
                                     bias=1e-30)
                norm_pending.append((lh, od, dd, den))
            finish_normalize()
            return at_t

        def emit_wo(qg, at_t):
            for ts in range(4):
                for eb in range(NTG):
                    wps = singp.tile([128, TG], F32, tag="sg", name=f"w{qg}_{ts}_{eb}")
                    for lh in range(4):
                        nc.tensor.matmul(wps[:], at_t[lh][:, ts * 128:(ts + 1) * 128],
                                         wo_t[lh][:, eb * TG:(eb + 1) * TG],
                                         start=(lh == 0), stop=(lh == 3))
                    ob = obp.tile([128, TG], F32, tag="ob", name=f"ob{qg}_{ts}_{eb}")
                    nc.scalar.copy(ob[:], wps[:])
                    nc.sync.dma_start(
                        out[qg * TG + ts * 128:qg * TG + (ts + 1) * 128,
                            eb * TG:(eb + 1) * TG], ob[:])

        # schedule: proj0 attn0 proj1 wo0 attn1 proj2 wo1 attn2 proj3 wo2 attn3 wo3
        qrot = emit_proj(0)
        for i in range(nmt):
            nc.sync.dma_start(mt_t[i][:], mtd[i, :, :])
        for lh in range(4):
            nc.sync.dma_start(wo_t[lh][:], wog[lh * 128:(lh + 1) * 128, :])
        for tg in range(NTG):
            at_t = emit_attention(tg, qrot)
            if tg + 1 < NTG:
                qrot = emit_proj(tg + 1)
            emit_wo(tg, at_t)

    nc.compile()
    return nc


def _host_prep(x, mask, wq, wk, wv, wo):
    m2dm = np.asarray(mask).reshape(T, T) != 0
    cfg = []
    mask_tiles = []
    mask_key = {}
    for qg in range(NTG):
        qs = slice(qg * TG, (qg + 1) * TG)
        lst = []
        for c in range(T // 128):
            sub = m2dm[qs, c * 128:(c + 1) * 128]
            if not sub.any():
                continue
            if sub.all():
                lst.append((c, -1))
            else:
                tilea = np.ascontiguousarray(sub.T).astype(np.float32)
                key = tilea.tobytes()
                if key not in mask_key:
                    mask_key[key] = len(mask_tiles)
                    mask_tiles.append(tilea)
                lst.append((c, mask_key[key]))
        cfg.append(tuple(lst))
    cfg = tuple(cfg)
    if mask_tiles:
        mt = np.stack(mask_tiles, 0)
    else:
        mt = np.zeros((1, 128, TG), np.float32)

    inv = 1.0 / (10000.0 ** (np.arange(64, dtype=np.float64) / 64))
    ang = np.arange(T, dtype=np.float64)[:, None] * inv[None, :]
    c64 = np.cos(ang).astype(np.float32).T
    s64 = np.sin(ang).astype(np.float32).T
    c2 = np.ascontiguousarray(np.concatenate([c64, c64], 0))
    s2 = np.ascontiguousarray(np.concatenate([s64, s64], 0))
    ones = np.ones((128, 128), np.float32)

    in_maps = []
    for b in range(B):
        xTb = np.ascontiguousarray(x[b].T)
        for g in range(4):
            heads = [2 * g, 2 * g + 1, 2 * g + 8, 2 * g + 9]
            kvh = [2 * g, 2 * g + 1]
            wq_a = np.empty((E, 4 * D), np.float32)
            for lh, h in enumerate(heads):
                wq_a[:, lh * D:lh * D + 64] = wq[:, h * D:(h + 1) * D:2]
                wq_a[:, lh * D + 64:(lh + 1) * D] = wq[:, h * D + 1:(h + 1) * D:2]
            wk_a = np.empty((E, 2 * D), np.float32)
            for lk, h in enumerate(kvh):
                wk_a[:, lk * D:lk * D + 64] = wk[:, h * D:(h + 1) * D:2]
                wk_a[:, lk * D + 64:(lk + 1) * D] = wk[:, h * D + 1:(h + 1) * D:2]
            wv_g = np.ascontiguousarray(wv[:, kvh[0] * D:(kvh[0] + 2) * D])
            wo_g = np.ascontiguousarray(
                np.concatenate([wo[h * D:(h + 1) * D] for h in heads], 0))
            in_maps.append({
                "xT": xTb, "wqa": wq_a, "wka": wk_a, "wvg": wv_g, "wog": wo_g,
                "c2": c2, "s2": s2, "ones": ones, "mt": mt,
            })
    return cfg, in_maps


def kernel(x, mask, wq, wk, wv, wo, _profile=None):
    x = np.ascontiguousarray(np.asarray(x, dtype=np.float32))
    wq = np.asarray(wq, dtype=np.float32)
    wk = np.asarray(wk, dtype=np.float32)
    wv = np.asarray(wv, dtype=np.float32)
    wo = np.asarray(wo, dtype=np.float32)
    cfg, in_maps = _host_prep(x, mask, wq, wk, wv, wo)
    if cfg not in _cache:
        _cache[cfg] = _build_program(cfg)
    nc = _cache[cfg]
    kwargs = dict(_profile) if _profile else {}
    res = run_bass_kernel_spmd(nc, in_maps, core_ids=list(range(8)), **kwargs)
    if _profile is not None and isinstance(_profile, dict):
        _profile["result"] = res
    outp = np.zeros((B, T, E), np.float32)
    for b in range(B):
        for g in range(4):
            outp[b] += res.results[b * 4 + g]["o"]
    return outp


# revision 17
# speedup vs baseline: 1.6943x; 1.0010x over previous
"""GQA attention (B=2,T=2048,E=2048,H=16,Hkv=8,D=128) with RoPE + causal mask,
sharded over 8 NeuronCores: core = (batch b, head-group g) with q-heads
{2g,2g+1,2g+8,2g+9} and kv-heads {2g,2g+1}. Each core computes its 4 heads'
attention for the full sequence plus its partial output projection; the host
sums the 4 partials per batch.

Device layout notes:
 - All matmuls run as float32r (full-rate fp32 on PE for moving dim >= 256).
 - Q/K are produced directly in [d, t] (transposed) layout with even/odd
   RoPE deinterleave folded into host-permuted weight columns; RoPE rotation
   is 4 DVE ops per [128,512] tile using cross-base-partition reads (one
   operand kept in PSUM, which the BIR verifier allows).
 - Scores are computed as S^T [kv,q] so no P transposes are needed; softmax
   denominators come from a ones-column matmul; normalization is applied to
   the [D,q] attention output via a K=1 broadcast matmul + DVE multiply.
"""
import sys
if "/opt/trn_rl_repo" not in sys.path:
    sys.path.insert(0, "/opt/trn_rl_repo")
from contextlib import ExitStack

import numpy as np

import concourse.bass as bass
import concourse.tile as tile
from concourse import bacc, mybir
from concourse.bass_utils import run_bass_kernel_spmd

F32 = mybir.dt.float32
F32R = mybir.dt.float32r
EXP = mybir.ActivationFunctionType.Exp

B, T, E = 2, 2048, 2048
H, H_KV, D = 16, 8, 128
TG = 512                 # q-group / moving-dim tile
NTG = T // TG            # 4
NEC = E // 128           # 16 contraction chunks for projections
SCALE = float(D) ** -0.5

_cache: dict = {}


def _build_program(cfg):
    """cfg: tuple per qg of tuple of (chunk_index, mask_tile_idx or -1)."""
    nmt = max(1, 1 + max((mi for qgc in cfg for _, mi in qgc), default=-1))
    nc = bacc.Bacc("TRN2", target_bir_lowering=False, debug=False, num_devices=8)

    xT = nc.dram_tensor("xT", [E, T], F32R, kind="ExternalInput").ap()
    wqa = nc.dram_tensor("wqa", [E, 4 * D], F32R, kind="ExternalInput").ap()
    wka = nc.dram_tensor("wka", [E, 2 * D], F32R, kind="ExternalInput").ap()
    wvg = nc.dram_tensor("wvg", [E, 2 * D], F32R, kind="ExternalInput").ap()
    wog = nc.dram_tensor("wog", [4 * D, E], F32R, kind="ExternalInput").ap()
    c2d = nc.dram_tensor("c2", [128, T], F32, kind="ExternalInput").ap()
    s2d = nc.dram_tensor("s2", [128, T], F32, kind="ExternalInput").ap()
    onesd = nc.dram_tensor("ones", [128, 128], F32R, kind="ExternalInput").ap()
    mtd = nc.dram_tensor("mt", [nmt, 128, TG], F32R, kind="ExternalInput").ap()
    out = nc.dram_tensor("o", [T, E], F32, kind="ExternalOutput").ap()

    with tile.TileContext(nc) as tc, ExitStack() as ctx:
        cp = ctx.enter_context(tc.tile_pool(name="const", bufs=1))
        pairp = ctx.enter_context(tc.tile_pool(name="pairp", bufs=2, space="PSUM"))
        singp = ctx.enter_context(tc.tile_pool(name="singp", bufs=4, space="PSUM"))
        xp = ctx.enter_context(tc.tile_pool(name="xp", bufs=4))
        csp = ctx.enter_context(tc.tile_pool(name="csp", bufs=2))
        m2p = ctx.enter_context(tc.tile_pool(name="m2p", bufs=2))
        ptp = ctx.enter_context(tc.tile_pool(name="ptp", bufs=4))
        atp = ctx.enter_context(tc.tile_pool(name="atp", bufs=1))
        obp = ctx.enter_context(tc.tile_pool(name="obp", bufs=2))
        rcp = ctx.enter_context(tc.tile_pool(name="rcp", bufs=2))

        # ---- resident constants (heavy DMAs deferred for fast rampup) ----
        wq_t, wk_t, wv_t = [], [], []
        for e in range(NEC):
            wq_t.append(cp.tile([128, 4 * D], F32R, tag=f"wq{e}", name=f"wq{e}"))
            wk_t.append(cp.tile([128, 2 * D], F32R, tag=f"wk{e}", name=f"wk{e}"))
            wv_t.append(cp.tile([128, 2 * D], F32R, tag=f"wv{e}", name=f"wv{e}"))
        wo_t = [cp.tile([128, E], F32R, tag=f"wo{lh}", name=f"wo{lh}") for lh in range(4)]
        ones_col = cp.tile([128, 1], F32R, tag="ones_col", name="ones_col")
        nc.sync.dma_start(ones_col[:], onesd[:, 0:1])
        ones_row = cp.tile([1, 128], F32R, tag="ones_row", name="ones_row")
        nc.sync.dma_start(ones_row[:], onesd[0:1, :])
        mt_t = [cp.tile([128, TG], F32R, tag=f"mt{i}", name=f"mt{i}") for i in range(nmt)]
        # K^T (rotated) and V persist for the whole sequence
        krot = [cp.tile([128, T], F32R, tag=f"kr{lk}", name=f"kr{lk}") for lk in range(2)]
        vres = [cp.tile([128, 2 * D], F32R, tag=f"v{t}", name=f"v{t}") for t in range(T // 128)]

        def emit_proj(tg):
            """Single-pass projections for t-group tg: Q^T(4 heads), K^T(2),
            V(4 t-subtiles as 2 bank-paired accumulators)."""
            tgs = slice(tg * TG, (tg + 1) * TG)
            c2t = csp.tile([128, TG], F32, tag="c2t", name=f"c2t{tg}")
            nc.sync.dma_start(c2t[:], c2d[:, tgs])
            s2t = csp.tile([128, TG], F32, tag="s2t", name=f"s2t{tg}")
            nc.sync.dma_start(s2t[:], s2d[:, tgs])

            qp2 = [pairp.tile([128, 2 * TG], F32, tag="pp", name=f"qp{i}_{tg}") for i in range(2)]
            qp = [qp2[lh // 2][:, (lh % 2) * TG:(lh % 2 + 1) * TG] for lh in range(4)]
            kp = [singp.tile([128, TG], F32, tag="sg", name=f"kp{lk}_{tg}") for lk in range(2)]
            vp = [singp.tile([128, TG], F32, tag="sg", name=f"vp{tsp}_{tg}") for tsp in range(2)]
            for e in range(NEC):
                if tg == 0:
                    nc.sync.dma_start(wq_t[e][:], wqa[e * 128:(e + 1) * 128, :])
                xt = xp.tile([128, TG], F32R, tag="xt", name=f"xt{tg}_{e}")
                nc.sync.dma_start(xt[:], xT[e * 128:(e + 1) * 128, tgs])
                if tg == 0:
                    nc.sync.dma_start(wk_t[e][:], wka[e * 128:(e + 1) * 128, :])
                    nc.sync.dma_start(wv_t[e][:], wvg[e * 128:(e + 1) * 128, :])
                for lh in range(4):
                    nc.tensor.matmul(qp[lh], wq_t[e][:, lh * D:(lh + 1) * D], xt[:],
                                     start=(e == 0), stop=(e == NEC - 1))
                for lk in range(2):
                    nc.tensor.matmul(kp[lk][:], wk_t[e][:, lk * D:(lk + 1) * D], xt[:],
                                     start=(e == 0), stop=(e == NEC - 1))
                for tsp in range(2):
                    for k in range(2):
                        ts = tsp * 2 + k
                        nc.tensor.matmul(vp[tsp][:, k * 256:(k + 1) * 256],
                                         xt[:, ts * 128:(ts + 1) * 128], wv_t[e][:],
                                         start=(e == 0 and k == 0), stop=(e == NEC - 1),
                                         skip_group_check=True)

            # RoPE rotation (4 DVE ops per [128,512] tile, in-place in PSUM)
            qrot = [atp.tile([128, TG], F32R, tag=f"qr{lh}", name=f"qr{lh}_{tg}")
                    for lh in range(4)]

            def rope(dst, src):
                m2 = m2p.tile([128, TG], F32, tag="m2")
                nc.vector.tensor_mul(m2[:], src, s2t[:])
                nc.vector.tensor_mul(src, src, c2t[:])
                nc.vector.tensor_sub(dst[0:64, :], src[0:64, :], m2[64:128, :])
                nc.vector.tensor_add(dst[64:128, :], src[64:128, :], m2[0:64, :])

            for tsp in range(2):
                for k in range(2):
                    ts = tsp * 2 + k
                    nc.scalar.copy(vres[tg * 4 + ts][:], vp[tsp][:, k * 256:(k + 1) * 256])
            rope(krot[0][:, tgs], kp[0][:])
            rope(qrot[0][:], qp[0])
            rope(krot[1][:, tgs], kp[1][:])
            for lh in range(1, 4):
                rope(qrot[lh][:], qp[lh])
            return qrot

        def emit_attention(qg, qrot):
            """Attention for q-group qg. Returns the 4 normalized attnT tiles."""
            chunks = cfg[qg]
            nck = len(chunks)
            at_t = [atp.tile([128, TG], F32R, tag=f"at{lh}", name=f"at{qg}_{lh}")
                    for lh in range(4)]
            norm_pending = []

            def finish_normalize():
                if not norm_pending:
                    return
                lh, od, dd, den = norm_pending.pop()
                bt = singp.tile([128, TG], F32, tag="sg", name=f"bt{qg}_{lh}")
                nc.tensor.matmul(bt[:], ones_row[:], den[:], start=True, stop=True)
                rec = rcp.tile([128, TG], F32, tag="rec", name=f"rec{qg}_{lh}")
                nc.vector.reciprocal_approx_fast(rec[:], bt[:])
                with nc.allow_low_precision(reason="fp32r attn output for wo matmul"):
                    nc.vector.tensor_mul(at_t[lh][:], od[:], rec[:])

            if nck == 0:
                for lh in range(4):
                    nc.vector.memset(at_t[lh][:], 0.0)
                return at_t

            pairs = [chunks[i:i + 2] for i in range(0, nck, 2)]
            for lh in range(4):
                lk = lh % 2
                od = singp.tile([128, TG], F32, tag="sg", name=f"od{qg}_{lh}")
                dd = singp.tile([1, TG], F32, tag="sg", name=f"dd{qg}_{lh}")
                pts = []

                def emit_av_pair(pi):
                    pt, pair = pts[pi]
                    for k, (c, mi) in enumerate(pair):
                        idx = 2 * pi + k
                        nc.tensor.matmul(od[:], vres[c][:, lk * D:(lk + 1) * D],
                                         pt[:, k * TG:(k + 1) * TG],
                                         start=(idx == 0), stop=(idx == nck - 1),
                                         skip_group_check=True)
                        nc.tensor.matmul(dd[:], ones_col[:],
                                         pt[:, k * TG:(k + 1) * TG],
                                         start=(idx == 0), stop=(idx == nck - 1),
                                         skip_group_check=True)

                pending = []
                for pi, pair in enumerate(pairs):
                    sp = pairp.tile([128, 2 * TG], F32, tag="pp", name=f"sp{qg}_{lh}_{pi}")
                    for k, (c, mi) in enumerate(pair):
                        nc.tensor.matmul(sp[:, k * TG:(k + 1) * TG],
                                         krot[lk][:, c * 128:(c + 1) * 128],
                                         qrot[lh][:], start=True, stop=True)
                    pt = ptp.tile([128, 2 * TG], F32R, tag="pt", name=f"pt{qg}_{lh}_{pi}")
                    w = len(pair) * TG
                    nc.scalar.activation(pt[:, 0:w], sp[:, 0:w], EXP, scale=SCALE)
                    for k, (c, mi) in enumerate(pair):
                        if mi >= 0:
                            nc.vector.tensor_mul(pt[:, k * TG:(k + 1) * TG],
                                                 pt[:, k * TG:(k + 1) * TG], mt_t[mi][:])
                    pts.append((pt, pair))
                    pending.append(pi)
                    if pi == 1:
                        finish_normalize()   # previous head, hidden under refill
                    if len(pending) > 1:
                        emit_av_pair(pending.pop(0))
                if len(pairs) < 2:
                    finish_normalize()
                for pi in pending:
                    emit_av_pair(pi)
                den = rcp.tile([1, TG], F32R, tag="den", name=f"den{qg}_{lh}")
                nc.scalar.activation(den[:], dd[:], mybir.ActivationFunctionType.Copy,
                                     bias=1e-30)
                norm_pending.append((lh, od, dd, den))
            finish_normalize()
            return at_t

        def emit_wo(qg, at_t):
            for ts in range(4):
                for eb in range(NTG):
                    wps = singp.tile([128, TG], F32, tag="sg", name=f"w{qg}_{ts}_{eb}")
                    for lh in range(4):
                        nc.tensor.matmul(wps[:], at_t[lh][:, ts * 128:(ts + 1) * 128],
                                         wo_t[lh][:, eb * TG:(eb + 1) * TG],
                                         start=(lh == 0), stop=(lh == 3))
                    ob = obp.tile([128, TG], F32, tag="ob", name=f"ob{qg}_{ts}_{eb}")
                    nc.scalar.copy(ob[:], wps[:])
                    nc.sync.dma_start(
                        out[qg * TG + ts * 128:qg * TG + (ts + 1) * 128,
                            eb * TG:(eb + 1) * TG], ob[:])

        # schedule: proj0 attn0 proj1 wo0 attn1 proj2 wo1 attn2 proj3 wo2 attn3 wo3
        qrot = emit_proj(0)
        for i in range(nmt):
            nc.sync.dma_start(mt_t[i][:], mtd[i, :, :])
        for lh in range(4):
            nc.sync.dma_start(wo_t[lh][:], wog[lh * 128:(lh + 1) * 128, :])
        for tg in range(NTG):
            at_t = emit_attention(tg, qrot)
            if tg + 1 < NTG:
                qrot = emit_proj(tg + 1)
            emit_wo(tg, at_t)

    nc.compile()
    return nc


def _host_prep(x, mask, wq, wk, wv, wo):
    m2dm = np.asarray(mask).reshape(T, T) != 0
    cfg = []
    mask_tiles = []
    mask_key = {}
    for qg in range(NTG):
        qs = slice(qg * TG, (qg + 1) * TG)
        lst = []
        for c in range(T // 128):
            sub = m2dm[qs, c * 128:(c + 1) * 128]
            if not sub.any():
                continue
            if sub.all():
                lst.append((c, -1))
            else:
                tilea = np.ascontiguousarray(sub.T).astype(np.float32)
                key = tilea.tobytes()
                if key not in mask_key:
                    mask_key[key] = len(mask_tiles)
                    mask_tiles.append(tilea)
                lst.append((c, mask_key[key]))
        cfg.append(tuple(lst))
    cfg = tuple(cfg)
    if mask_tiles:
        mt = np.stack(mask_tiles, 0)
    else:
        mt = np.zeros((1, 128, TG), np.float32)

    inv = 1.0 / (10000.0 ** (np.arange(64, dtype=np.float64) / 64))
    ang = np.arange(T, dtype=np.float64)[:, None] * inv[None, :]
    c64 = np.cos(ang).astype(np.float32).T
    s64 = np.sin(ang).astype(np.float32).T
    c2 = np.ascontiguousarray(np.concatenate([c64, c64], 0))
    s2 = np.ascontiguousarray(np.concatenate([s64, s64], 0))
    ones = np.ones((128, 128), np.float32)

    in_maps = []
    for b in range(B):
        xTb = np.ascontiguousarray(x[b].T)
        for g in range(4):
            heads = [2 * g, 2 * g + 1, 2 * g + 8, 2 * g + 9]
            kvh = [2 * g, 2 * g + 1]
            wq_a = np.empty((E, 4 * D), np.float32)
            for lh, h in enumerate(heads):
                wq_a[:, lh * D:lh * D + 64] = wq[:, h * D:(h + 1) * D:2]
                wq_a[:, lh * D + 64:(lh + 1) * D] = wq[:, h * D + 1:(h + 1) * D:2]
            wk_a = np.empty((E, 2 * D), np.float32)
            for lk, h in enumerate(kvh):
                wk_a[:, lk * D:lk * D + 64] = wk[:, h * D:(h + 1) * D:2]
                wk_a[:, lk * D + 64:(lk + 1) * D] = wk[:, h * D + 1:(h + 1) * D:2]
            wv_g = np.ascontiguousarray(wv[:, kvh[0] * D:(kvh[0] + 2) * D])
            wo_g = np.ascontiguousarray(
                np.concatenate([wo[h * D:(h + 1) * D] for h in heads], 0))
            in_maps.append({
                "xT": xTb, "wqa": wq_a, "wka": wk_a, "wvg": wv_g, "wog": wo_g,
                "c2": c2, "s2": s2, "ones": ones, "mt": mt,
            })
    return cfg, in_maps


def kernel(x, mask, wq, wk, wv, wo, _profile=None):
    x = np.ascontiguousarray(np.asarray(x, dtype=np.float32))
    wq = np.asarray(wq, dtype=np.float32)
    wk = np.asarray(wk, dtype=np.float32)
    wv = np.asarray(wv, dtype=np.float32)
    wo = np.asarray(wo, dtype=np.float32)
    cfg, in_maps = _host_prep(x, mask, wq, wk, wv, wo)
    if cfg not in _cache:
        _cache[cfg] = _build_program(cfg)
    nc = _cache[cfg]
    kwargs = dict(_profile) if _profile else {}
    res = run_bass_kernel_spmd(nc, in_maps, core_ids=list(range(8)), **kwargs)
    if _profile is not None and isinstance(_profile, dict):
        _profile["result"] = res
    outp = np.zeros((B, T, E), np.float32)
    for b in range(B):
        for g in range(4):
            outp[b] += res.results[b * 4 + g]["o"]
    return outp


# revision 18
# speedup vs baseline: 1.7027x; 1.0050x over previous
"""GQA attention (B=2,T=2048,E=2048,H=16,Hkv=8,D=128) with RoPE + causal mask,
sharded over 8 NeuronCores: core = (batch b, head-group g) with q-heads
{2g,2g+1,2g+8,2g+9} and kv-heads {2g,2g+1}. Each core computes its 4 heads'
attention for the full sequence plus its partial output projection; the host
sums the 4 partials per batch.

Device layout notes:
 - All matmuls run as float32r (full-rate fp32 on PE for moving dim >= 256).
 - Q/K are produced directly in [d, t] (transposed) layout with even/odd
   RoPE deinterleave folded into host-permuted weight columns; RoPE rotation
   is 4 DVE ops per [128,512] tile using cross-base-partition reads (one
   operand kept in PSUM, which the BIR verifier allows).
 - Scores are computed as S^T [kv,q] so no P transposes are needed; softmax
   denominators come from a ones-column matmul; normalization is applied to
   the [D,q] attention output via a K=1 broadcast matmul + DVE multiply.
"""
import sys
if "/opt/trn_rl_repo" not in sys.path:
    sys.path.insert(0, "/opt/trn_rl_repo")
from contextlib import ExitStack

import numpy as np

import concourse.bass as bass
import concourse.tile as tile
from concourse import bacc, mybir
from concourse.bass_utils import run_bass_kernel_spmd

F32 = mybir.dt.float32
F32R = mybir.dt.float32r
EXP = mybir.ActivationFunctionType.Exp

B, T, E = 2, 2048, 2048
H, H_KV, D = 16, 8, 128
TG = 512                 # q-group / moving-dim tile
NTG = T // TG            # 4
NEC = E // 128           # 16 contraction chunks for projections
SCALE = float(D) ** -0.5

_cache: dict = {}


def _build_program(cfg):
    """cfg: tuple per qg of tuple of (chunk_index, mask_tile_idx or -1)."""
    nmt = max(1, 1 + max((mi for qgc in cfg for _, mi in qgc), default=-1))
    nc = bacc.Bacc("TRN2", target_bir_lowering=False, debug=False, num_devices=8)

    xT = nc.dram_tensor("xT", [E, T], F32R, kind="ExternalInput").ap()
    wqa = nc.dram_tensor("wqa", [E, 4 * D], F32R, kind="ExternalInput").ap()
    wka = nc.dram_tensor("wka", [E, 2 * D], F32R, kind="ExternalInput").ap()
    wvg = nc.dram_tensor("wvg", [E, 2 * D], F32R, kind="ExternalInput").ap()
    wog = nc.dram_tensor("wog", [4 * D, E], F32R, kind="ExternalInput").ap()
    c2d = nc.dram_tensor("c2", [128, T], F32, kind="ExternalInput").ap()
    s2d = nc.dram_tensor("s2", [128, T], F32, kind="ExternalInput").ap()
    onesd = nc.dram_tensor("ones", [128, 128], F32R, kind="ExternalInput").ap()
    mtd = nc.dram_tensor("mt", [nmt, 128, TG], F32R, kind="ExternalInput").ap()
    out = nc.dram_tensor("o", [T, E], F32, kind="ExternalOutput").ap()
    causal_compat = all(
        (max(c for c, _ in qgc) * 128) // TG <= qg
        for qg, qgc in enumerate(cfg) if qgc)
    mask_resident = nmt <= 4
    qscr = None
    if not causal_compat:
        qscr = nc.dram_tensor("qscr", [NTG, 4, 128, TG], F32R, kind="Internal").ap()

    with tile.TileContext(nc) as tc, ExitStack() as ctx:
        cp = ctx.enter_context(tc.tile_pool(name="const", bufs=1))
        pairp = ctx.enter_context(tc.tile_pool(name="pairp", bufs=2, space="PSUM"))
        singp = ctx.enter_context(tc.tile_pool(name="singp", bufs=4, space="PSUM"))
        xp = ctx.enter_context(tc.tile_pool(name="xp", bufs=4))
        csp = ctx.enter_context(tc.tile_pool(name="csp", bufs=2))
        m2p = ctx.enter_context(tc.tile_pool(name="m2p", bufs=2))
        ptp = ctx.enter_context(tc.tile_pool(name="ptp", bufs=4))
        atp = ctx.enter_context(tc.tile_pool(name="atp", bufs=1))
        obp = ctx.enter_context(tc.tile_pool(name="obp", bufs=2))
        rcp = ctx.enter_context(tc.tile_pool(name="rcp", bufs=2))

        # ---- resident constants (heavy DMAs deferred for fast rampup) ----
        wq_t, wk_t, wv_t = [], [], []
        for e in range(NEC):
            wq_t.append(cp.tile([128, 4 * D], F32R, tag=f"wq{e}", name=f"wq{e}"))
            wk_t.append(cp.tile([128, 2 * D], F32R, tag=f"wk{e}", name=f"wk{e}"))
            wv_t.append(cp.tile([128, 2 * D], F32R, tag=f"wv{e}", name=f"wv{e}"))
        wo_t = [cp.tile([128, E], F32R, tag=f"wo{lh}", name=f"wo{lh}") for lh in range(4)]
        ones_col = cp.tile([128, 1], F32R, tag="ones_col", name="ones_col")
        nc.sync.dma_start(ones_col[:], onesd[:, 0:1])
        ones_row = cp.tile([1, 128], F32R, tag="ones_row", name="ones_row")
        nc.sync.dma_start(ones_row[:], onesd[0:1, :])
        mt_t = [cp.tile([128, TG], F32R, tag=f"mt{i}", name=f"mt{i}")
                for i in range(nmt if mask_resident else 0)]
        msp = ctx.enter_context(tc.tile_pool(name="msp", bufs=2))
        # K^T (rotated) and V persist for the whole sequence
        krot = [cp.tile([128, T], F32R, tag=f"kr{lk}", name=f"kr{lk}") for lk in range(2)]
        vres = [cp.tile([128, 2 * D], F32R, tag=f"v{t}", name=f"v{t}") for t in range(T // 128)]

        def emit_proj(tg):
            """Single-pass projections for t-group tg: Q^T(4 heads), K^T(2),
            V(4 t-subtiles as 2 bank-paired accumulators)."""
            tgs = slice(tg * TG, (tg + 1) * TG)
            c2t = csp.tile([128, TG], F32, tag="c2t", name=f"c2t{tg}")
            nc.sync.dma_start(c2t[:], c2d[:, tgs])
            s2t = csp.tile([128, TG], F32, tag="s2t", name=f"s2t{tg}")
            nc.sync.dma_start(s2t[:], s2d[:, tgs])

            qp2 = [pairp.tile([128, 2 * TG], F32, tag="pp", name=f"qp{i}_{tg}") for i in range(2)]
            qp = [qp2[lh // 2][:, (lh % 2) * TG:(lh % 2 + 1) * TG] for lh in range(4)]
            kp = [singp.tile([128, TG], F32, tag="sg", name=f"kp{lk}_{tg}") for lk in range(2)]
            vp = [singp.tile([128, TG], F32, tag="sg", name=f"vp{tsp}_{tg}") for tsp in range(2)]
            for e in range(NEC):
                if tg == 0:
                    nc.sync.dma_start(wq_t[e][:], wqa[e * 128:(e + 1) * 128, :])
                xt = xp.tile([128, TG], F32R, tag="xt", name=f"xt{tg}_{e}")
                nc.sync.dma_start(xt[:], xT[e * 128:(e + 1) * 128, tgs])
                if tg == 0:
                    nc.sync.dma_start(wk_t[e][:], wka[e * 128:(e + 1) * 128, :])
                    nc.sync.dma_start(wv_t[e][:], wvg[e * 128:(e + 1) * 128, :])
                for lh in range(4):
                    nc.tensor.matmul(qp[lh], wq_t[e][:, lh * D:(lh + 1) * D], xt[:],
                                     start=(e == 0), stop=(e == NEC - 1))
                for lk in range(2):
                    nc.tensor.matmul(kp[lk][:], wk_t[e][:, lk * D:(lk + 1) * D], xt[:],
                                     start=(e == 0), stop=(e == NEC - 1))
                for tsp in range(2):
                    for k in range(2):
                        ts = tsp * 2 + k
                        nc.tensor.matmul(vp[tsp][:, k * 256:(k + 1) * 256],
                                         xt[:, ts * 128:(ts + 1) * 128], wv_t[e][:],
                                         start=(e == 0 and k == 0), stop=(e == NEC - 1),
                                         skip_group_check=True)

            # RoPE rotation (4 DVE ops per [128,512] tile, in-place in PSUM)
            qrot = [atp.tile([128, TG], F32R, tag=f"qr{lh}", name=f"qr{lh}_{tg}")
                    for lh in range(4)]

            def rope(dst, src):
                m2 = m2p.tile([128, TG], F32, tag="m2")
                nc.vector.tensor_mul(m2[:], src, s2t[:])
                nc.vector.tensor_mul(src, src, c2t[:])
                nc.vector.tensor_sub(dst[0:64, :], src[0:64, :], m2[64:128, :])
                nc.vector.tensor_add(dst[64:128, :], src[64:128, :], m2[0:64, :])

            for tsp in range(2):
                for k in range(2):
                    ts = tsp * 2 + k
                    nc.scalar.copy(vres[tg * 4 + ts][:], vp[tsp][:, k * 256:(k + 1) * 256])
            rope(krot[0][:, tgs], kp[0][:])
            rope(qrot[0][:], qp[0])
            rope(krot[1][:, tgs], kp[1][:])
            for lh in range(1, 4):
                rope(qrot[lh][:], qp[lh])
            if qscr is not None:
                for lh in range(4):
                    nc.sync.dma_start(qscr[tg, lh, :, :], qrot[lh][:])
            return qrot

        def emit_attention(qg, qrot):
            """Attention for q-group qg. Returns the 4 normalized attnT tiles."""
            chunks = cfg[qg]
            nck = len(chunks)
            at_t = [atp.tile([128, TG], F32R, tag=f"at{lh}", name=f"at{qg}_{lh}")
                    for lh in range(4)]
            norm_pending = []

            def finish_normalize():
                if not norm_pending:
                    return
                lh, od, dd, den = norm_pending.pop()
                bt = singp.tile([128, TG], F32, tag="sg", name=f"bt{qg}_{lh}")
                nc.tensor.matmul(bt[:], ones_row[:], den[:], start=True, stop=True)
                rec = rcp.tile([128, TG], F32, tag="rec", name=f"rec{qg}_{lh}")
                nc.vector.reciprocal_approx_fast(rec[:], bt[:])
                with nc.allow_low_precision(reason="fp32r attn output for wo matmul"):
                    nc.vector.tensor_mul(at_t[lh][:], od[:], rec[:])

            if nck == 0:
                for lh in range(4):
                    nc.vector.memset(at_t[lh][:], 0.0)
                return at_t

            pairs = [chunks[i:i + 2] for i in range(0, nck, 2)]
            for lh in range(4):
                lk = lh % 2
                od = singp.tile([128, TG], F32, tag="sg", name=f"od{qg}_{lh}")
                dd = singp.tile([1, TG], F32, tag="sg", name=f"dd{qg}_{lh}")
                pts = []

                def emit_av_pair(pi):
                    pt, pair = pts[pi]
                    for k, (c, mi) in enumerate(pair):
                        idx = 2 * pi + k
                        nc.tensor.matmul(od[:], vres[c][:, lk * D:(lk + 1) * D],
                                         pt[:, k * TG:(k + 1) * TG],
                                         start=(idx == 0), stop=(idx == nck - 1),
                                         skip_group_check=True)
                        nc.tensor.matmul(dd[:], ones_col[:],
                                         pt[:, k * TG:(k + 1) * TG],
                                         start=(idx == 0), stop=(idx == nck - 1),
                                         skip_group_check=True)

                pending = []
                for pi, pair in enumerate(pairs):
                    sp = pairp.tile([128, 2 * TG], F32, tag="pp", name=f"sp{qg}_{lh}_{pi}")
                    for k, (c, mi) in enumerate(pair):
                        nc.tensor.matmul(sp[:, k * TG:(k + 1) * TG],
                                         krot[lk][:, c * 128:(c + 1) * 128],
                                         qrot[lh][:], start=True, stop=True)
                    pt = ptp.tile([128, 2 * TG], F32R, tag="pt", name=f"pt{qg}_{lh}_{pi}")
                    w = len(pair) * TG
                    nc.scalar.activation(pt[:, 0:w], sp[:, 0:w], EXP, scale=SCALE)
                    for k, (c, mi) in enumerate(pair):
                        if mi >= 0:
                            if mask_resident:
                                mtile = mt_t[mi][:]
                            else:
                                mst = msp.tile([128, TG], F32R, tag="ms",
                                               name=f"ms{qg}_{lh}_{pi}_{k}")
                                nc.sync.dma_start(mst[:], mtd[mi, :, :])
                                mtile = mst[:]
                            nc.vector.tensor_mul(pt[:, k * TG:(k + 1) * TG],
                                                 pt[:, k * TG:(k + 1) * TG], mtile)
                    pts.append((pt, pair))
                    pending.append(pi)
                    if pi == 1:
                        finish_normalize()   # previous head, hidden under refill
                    if len(pending) > 1:
                        emit_av_pair(pending.pop(0))
                if len(pairs) < 2:
                    finish_normalize()
                for pi in pending:
                    emit_av_pair(pi)
                den = rcp.tile([1, TG], F32R, tag="den", name=f"den{qg}_{lh}")
                nc.scalar.activation(den[:], dd[:], mybir.ActivationFunctionType.Copy,
                                     bias=1e-30)
                norm_pending.append((lh, od, dd, den))
            finish_normalize()
            return at_t

        def emit_wo(qg, at_t):
            for ts in range(4):
                for eb in range(NTG):
                    wps = singp.tile([128, TG], F32, tag="sg", name=f"w{qg}_{ts}_{eb}")
                    for lh in range(4):
                        nc.tensor.matmul(wps[:], at_t[lh][:, ts * 128:(ts + 1) * 128],
                                         wo_t[lh][:, eb * TG:(eb + 1) * TG],
                                         start=(lh == 0), stop=(lh == 3))
                    ob = obp.tile([128, TG], F32, tag="ob", name=f"ob{qg}_{ts}_{eb}")
                    nc.scalar.copy(ob[:], wps[:])
                    nc.sync.dma_start(
                        out[qg * TG + ts * 128:qg * TG + (ts + 1) * 128,
                            eb * TG:(eb + 1) * TG], ob[:])

        if causal_compat:
            # schedule: proj0 attn0 proj1 wo0 attn1 proj2 wo1 attn2 proj3 wo2 attn3 wo3
            qrot = emit_proj(0)
            for i in range(len(mt_t)):
                nc.sync.dma_start(mt_t[i][:], mtd[i, :, :])
            for lh in range(4):
                nc.sync.dma_start(wo_t[lh][:], wog[lh * 128:(lh + 1) * 128, :])
            for tg in range(NTG):
                at_t = emit_attention(tg, qrot)
                if tg + 1 < NTG:
                    qrot = emit_proj(tg + 1)
                emit_wo(tg, at_t)
        else:
            # general masks: project everything (Q^T spilled to DRAM), then attend
            for i in range(len(mt_t)):
                nc.sync.dma_start(mt_t[i][:], mtd[i, :, :])
            for lh in range(4):
                nc.sync.dma_start(wo_t[lh][:], wog[lh * 128:(lh + 1) * 128, :])
            for tg in range(NTG):
                emit_proj(tg)
            for qg in range(NTG):
                qrot = [atp.tile([128, TG], F32R, tag=f"qr{lh}", name=f"ql{qg}_{lh}")
                        for lh in range(4)]
                for lh in range(4):
                    nc.sync.dma_start(qrot[lh][:], qscr[qg, lh, :, :])
                at_t = emit_attention(qg, qrot)
                emit_wo(qg, at_t)

    nc.compile()
    return nc


def _host_prep(x, mask, wq, wk, wv, wo):
    m2dm = np.asarray(mask).reshape(T, T) != 0
    cfg = []
    mask_tiles = []
    mask_key = {}
    for qg in range(NTG):
        qs = slice(qg * TG, (qg + 1) * TG)
        lst = []
        for c in range(T // 128):
            sub = m2dm[qs, c * 128:(c + 1) * 128]
            if not sub.any():
                continue
            if sub.all():
                lst.append((c, -1))
            else:
                tilea = np.ascontiguousarray(sub.T).astype(np.float32)
                key = tilea.tobytes()
                if key not in mask_key:
                    mask_key[key] = len(mask_tiles)
                    mask_tiles.append(tilea)
                lst.append((c, mask_key[key]))
        cfg.append(tuple(lst))
    cfg = tuple(cfg)
    if mask_tiles:
        mt = np.stack(mask_tiles, 0)
    else:
        mt = np.zeros((1, 128, TG), np.float32)

    inv = 1.0 / (10000.0 ** (np.arange(64, dtype=np.float64) / 64))
    ang = np.arange(T, dtype=np.float64)[:, None] * inv[None, :]
    c64 = np.cos(ang).astype(np.float32).T
    s64 = np.sin(ang).astype(np.float32).T
    c2 = np.ascontiguousarray(np.concatenate([c64, c64], 0))
    s2 = np.ascontiguousarray(np.concatenate([s64, s64], 0))
    ones = np.ones((128, 128), np.float32)

    in_maps = []
    for b in range(B):
        xTb = np.ascontiguousarray(x[b].T)
        for g in range(4):
            heads = [2 * g, 2 * g + 1, 2 * g + 8, 2 * g + 9]
            kvh = [2 * g, 2 * g + 1]
            wq_a = np.empty((E, 4 * D), np.float32)
            for lh, h in enumerate(heads):
                wq_a[:, lh * D:lh * D + 64] = wq[:, h * D:(h + 1) * D:2]
                wq_a[:, lh * D + 64:(lh + 1) * D] = wq[:, h * D + 1:(h + 1) * D:2]
            wk_a = np.empty((E, 2 * D), np.float32)
            for lk, h in enumerate(kvh):
                wk_a[:, lk * D:lk * D + 64] = wk[:, h * D:(h + 1) * D:2]
                wk_a[:, lk * D + 64:(lk + 1) * D] = wk[:, h * D + 1:(h + 1) * D:2]
            wv_g = np.ascontiguousarray(wv[:, kvh[0] * D:(kvh[0] + 2) * D])
            wo_g = np.ascontiguousarray(
                np.concatenate([wo[h * D:(h + 1) * D] for h in heads], 0))
            in_maps.append({
                "xT": xTb, "wqa": wq_a, "wka": wk_a, "wvg": wv_g, "wog": wo_g,
                "c2": c2, "s2": s2, "ones": ones, "mt": mt,
            })
    return cfg, in_maps


def kernel(x, mask, wq, wk, wv, wo, _profile=None):
    x = np.ascontiguousarray(np.asarray(x, dtype=np.float32))
    wq = np.asarray(wq, dtype=np.float32)
    wk = np.asarray(wk, dtype=np.float32)
    wv = np.asarray(wv, dtype=np.float32)
    wo = np.asarray(wo, dtype=np.float32)
    cfg, in_maps = _host_prep(x, mask, wq, wk, wv, wo)
    if cfg not in _cache:
        _cache[cfg] = _build_program(cfg)
    nc = _cache[cfg]
    kwargs = dict(_profile) if _profile else {}
    res = run_bass_kernel_spmd(nc, in_maps, core_ids=list(range(8)), **kwargs)
    if _profile is not None and isinstance(_profile, dict):
        _profile["result"] = res
    outp = np.zeros((B, T, E), np.float32)
    for b in range(B):
        for g in range(4):
            outp[b] += res.results[b * 4 + g]["o"]
    return outp


# revision 19
# speedup vs baseline: 1.8678x; 1.0970x over previous
"""GQA attention (B=2,T=2048,E=2048,H=16,Hkv=8,D=128) with RoPE + causal mask,
sharded over 8 NeuronCores: core = (batch b, head-group g) with q-heads
{2g,2g+1,2g+8,2g+9} and kv-heads {2g,2g+1}. Each core computes its 4 heads'
attention for the full sequence plus its partial output projection; the host
sums the 4 partials per batch.

Device layout notes:
 - All matmuls run as float32r (full-rate fp32 on PE for moving dim >= 256).
 - Q/K are produced directly in [d, t] (transposed) layout with even/odd
   RoPE deinterleave folded into host-permuted weight columns; RoPE rotation
   is 4 DVE ops per [128,512] tile using cross-base-partition reads (one
   operand kept in PSUM, which the BIR verifier allows).
 - Scores are computed as S^T [kv,q] so no P transposes are needed; softmax
   denominators come from a ones-column matmul; normalization is applied to
   the [D,q] attention output via a K=1 broadcast matmul + DVE multiply.
"""
import sys
if "/opt/trn_rl_repo" not in sys.path:
    sys.path.insert(0, "/opt/trn_rl_repo")
from contextlib import ExitStack

import numpy as np

import concourse.bass as bass
import concourse.tile as tile
from concourse import bacc, mybir
from concourse.bass_utils import run_bass_kernel_spmd

F32 = mybir.dt.float32
F32R = mybir.dt.float32r
EXP = mybir.ActivationFunctionType.Exp

B, T, E = 2, 2048, 2048
H, H_KV, D = 16, 8, 128
TG = 512                 # q-group / moving-dim tile
NTG = T // TG            # 4
NEC = E // 128           # 16 contraction chunks for projections
SCALE = float(D) ** -0.5

_cache: dict = {}


def _build_program(cfg):
    """cfg: tuple per qg of tuple of (chunk_index, mask_tile_idx or -1)."""
    nmt = max(1, 1 + max((mi for qgc in cfg for _, mi in qgc), default=-1))
    nc = bacc.Bacc("TRN2", target_bir_lowering=False, debug=False, num_devices=8)

    xT = nc.dram_tensor("xT", [E, T], F32R, kind="ExternalInput").ap()
    wqa = nc.dram_tensor("wqa", [E, 4 * D], F32R, kind="ExternalInput").ap()
    wka = nc.dram_tensor("wka", [E, 2 * D], F32R, kind="ExternalInput").ap()
    wvg = nc.dram_tensor("wvg", [E, 2 * D], F32R, kind="ExternalInput").ap()
    wog = nc.dram_tensor("wog", [4 * D, E], F32R, kind="ExternalInput").ap()
    c2d = nc.dram_tensor("c2", [128, T], F32, kind="ExternalInput").ap()
    s2d = nc.dram_tensor("s2", [128, T], F32, kind="ExternalInput").ap()
    onesd = nc.dram_tensor("ones", [128, 128], F32R, kind="ExternalInput").ap()
    mtd = nc.dram_tensor("mt", [nmt, 128, TG], F32R, kind="ExternalInput").ap()
    out = nc.dram_tensor("o", [T, E], F32, kind="ExternalOutput").ap()
    causal_compat = all(
        (max(c for c, _ in qgc) * 128) // TG <= qg
        for qg, qgc in enumerate(cfg) if qgc)
    mask_resident = nmt <= 4
    qscr = None
    if not causal_compat:
        qscr = nc.dram_tensor("qscr", [NTG, 4, 128, TG], F32R, kind="Internal").ap()

    with tile.TileContext(nc) as tc, ExitStack() as ctx:
        cp = ctx.enter_context(tc.tile_pool(name="const", bufs=1))
        pairp = ctx.enter_context(tc.tile_pool(name="pairp", bufs=2, space="PSUM"))
        singp = ctx.enter_context(tc.tile_pool(name="singp", bufs=4, space="PSUM"))
        xp = ctx.enter_context(tc.tile_pool(name="xp", bufs=5))
        csp = ctx.enter_context(tc.tile_pool(name="csp", bufs=2))
        m2p = ctx.enter_context(tc.tile_pool(name="m2p", bufs=2))
        ptp = ctx.enter_context(tc.tile_pool(name="ptp", bufs=4))
        atp = ctx.enter_context(tc.tile_pool(name="atp", bufs=1))
        obp = ctx.enter_context(tc.tile_pool(name="obp", bufs=3))
        rcp = ctx.enter_context(tc.tile_pool(name="rcp", bufs=2))

        # ---- resident constants (heavy DMAs deferred for fast rampup) ----
        wq_t, wk_t, wv_t = [], [], []
        for e in range(NEC):
            wq_t.append(cp.tile([128, 4 * D], F32R, tag=f"wq{e}", name=f"wq{e}"))
            wk_t.append(cp.tile([128, 2 * D], F32R, tag=f"wk{e}", name=f"wk{e}"))
            wv_t.append(cp.tile([128, 2 * D], F32R, tag=f"wv{e}", name=f"wv{e}"))
        wo_t = [cp.tile([128, E], F32R, tag=f"wo{lh}", name=f"wo{lh}") for lh in range(4)]
        ones_col = cp.tile([128, 1], F32R, tag="ones_col", name="ones_col")
        nc.sync.dma_start(ones_col[:], onesd[:, 0:1])
        ones_row = cp.tile([1, 128], F32R, tag="ones_row", name="ones_row")
        nc.sync.dma_start(ones_row[:], onesd[0:1, :])
        mt_t = [cp.tile([128, TG], F32R, tag=f"mt{i}", name=f"mt{i}")
                for i in range(nmt if mask_resident else 0)]
        msp = None if mask_resident else ctx.enter_context(tc.tile_pool(name="msp", bufs=2))
        # K^T (rotated) and V persist for the whole sequence
        krot = [cp.tile([128, T], F32R, tag=f"kr{lk}", name=f"kr{lk}") for lk in range(2)]
        vres = [cp.tile([128, 2 * D], F32R, tag=f"v{t}", name=f"v{t}") for t in range(T // 128)]

        def emit_proj(tg):
            """Single-pass projections for t-group tg: Q^T(4 heads), K^T(2),
            V(4 t-subtiles as 2 bank-paired accumulators)."""
            tgs = slice(tg * TG, (tg + 1) * TG)
            c2t = csp.tile([128, TG], F32, tag="c2t", name=f"c2t{tg}")
            nc.sync.dma_start(c2t[:], c2d[:, tgs])
            s2t = csp.tile([128, TG], F32, tag="s2t", name=f"s2t{tg}")
            nc.sync.dma_start(s2t[:], s2d[:, tgs])

            qp2 = [pairp.tile([128, 2 * TG], F32, tag="pp", name=f"qp{i}_{tg}") for i in range(2)]
            qp = [qp2[lh // 2][:, (lh % 2) * TG:(lh % 2 + 1) * TG] for lh in range(4)]
            kp = [singp.tile([128, TG], F32, tag="sg", name=f"kp{lk}_{tg}") for lk in range(2)]
            vp = [singp.tile([128, TG], F32, tag="sg", name=f"vp{tsp}_{tg}") for tsp in range(2)]
            for e in range(NEC):
                if tg == 0:
                    nc.sync.dma_start(wq_t[e][:], wqa[e * 128:(e + 1) * 128, :])
                xt = xp.tile([128, TG], F32R, tag="xt", name=f"xt{tg}_{e}")
                nc.sync.dma_start(xt[:], xT[e * 128:(e + 1) * 128, tgs])
                if tg == 0:
                    nc.sync.dma_start(wk_t[e][:], wka[e * 128:(e + 1) * 128, :])
                    nc.sync.dma_start(wv_t[e][:], wvg[e * 128:(e + 1) * 128, :])
                for lh in range(4):
                    nc.tensor.matmul(qp[lh], wq_t[e][:, lh * D:(lh + 1) * D], xt[:],
                                     start=(e == 0), stop=(e == NEC - 1))
                for lk in range(2):
                    nc.tensor.matmul(kp[lk][:], wk_t[e][:, lk * D:(lk + 1) * D], xt[:],
                                     start=(e == 0), stop=(e == NEC - 1))
                for tsp in range(2):
                    for k in range(2):
                        ts = tsp * 2 + k
                        nc.tensor.matmul(vp[tsp][:, k * 256:(k + 1) * 256],
                                         xt[:, ts * 128:(ts + 1) * 128], wv_t[e][:],
                                         start=(e == 0 and k == 0), stop=(e == NEC - 1),
                                         skip_group_check=True)

            # RoPE rotation (4 DVE ops per [128,512] tile, in-place in PSUM)
            qrot = [atp.tile([128, TG], F32R, tag=f"qr{lh}", name=f"qr{lh}_{tg}")
                    for lh in range(4)]

            def rope(dst, src):
                m2 = m2p.tile([128, TG], F32, tag="m2")
                nc.vector.tensor_mul(m2[:], src, s2t[:])
                nc.vector.tensor_mul(src, src, c2t[:])
                nc.vector.tensor_sub(dst[0:64, :], src[0:64, :], m2[64:128, :])
                nc.vector.tensor_add(dst[64:128, :], src[64:128, :], m2[0:64, :])

            for tsp in range(2):
                for k in range(2):
                    ts = tsp * 2 + k
                    nc.scalar.copy(vres[tg * 4 + ts][:], vp[tsp][:, k * 256:(k + 1) * 256])
            rope(krot[0][:, tgs], kp[0][:])
            rope(qrot[0][:], qp[0])
            rope(krot[1][:, tgs], kp[1][:])
            for lh in range(1, 4):
                rope(qrot[lh][:], qp[lh])
            if qscr is not None:
                for lh in range(4):
                    nc.sync.dma_start(qscr[tg, lh, :, :], qrot[lh][:])
            return qrot

        def emit_attention(qg, qrot):
            """Attention for q-group qg. Returns the 4 normalized attnT tiles."""
            chunks = cfg[qg]
            nck = len(chunks)
            at_t = [atp.tile([128, TG], F32R, tag=f"at{lh}", name=f"at{qg}_{lh}")
                    for lh in range(4)]
            norm_pending = []

            def finish_normalize():
                if not norm_pending:
                    return
                lh, od, dd, den = norm_pending.pop()
                bt = singp.tile([128, TG], F32, tag="sg", name=f"bt{qg}_{lh}")
                nc.tensor.matmul(bt[:], ones_row[:], den[:], start=True, stop=True)
                rec = rcp.tile([128, TG], F32, tag="rec", name=f"rec{qg}_{lh}")
                nc.vector.reciprocal_approx_fast(rec[:], bt[:])
                with nc.allow_low_precision(reason="fp32r attn output for wo matmul"):
                    nc.vector.tensor_mul(at_t[lh][:], od[:], rec[:])

            if nck == 0:
                for lh in range(4):
                    nc.vector.memset(at_t[lh][:], 0.0)
                return at_t

            pairs = [chunks[i:i + 2] for i in range(0, nck, 2)]
            for lh in range(4):
                lk = lh % 2
                od = singp.tile([128, TG], F32, tag="sg", name=f"od{qg}_{lh}")
                dd = singp.tile([1, TG], F32, tag="sg", name=f"dd{qg}_{lh}")
                pts = []

                def emit_av_pair(pi):
                    pt, pair = pts[pi]
                    for k, (c, mi) in enumerate(pair):
                        idx = 2 * pi + k
                        nc.tensor.matmul(od[:], vres[c][:, lk * D:(lk + 1) * D],
                                         pt[:, k * TG:(k + 1) * TG],
                                         start=(idx == 0), stop=(idx == nck - 1),
                                         skip_group_check=True)
                        nc.tensor.matmul(dd[:], ones_col[:],
                                         pt[:, k * TG:(k + 1) * TG],
                                         start=(idx == 0), stop=(idx == nck - 1),
                                         skip_group_check=True)

                pending = []
                for pi, pair in enumerate(pairs):
                    sp = pairp.tile([128, 2 * TG], F32, tag="pp", name=f"sp{qg}_{lh}_{pi}")
                    for k, (c, mi) in enumerate(pair):
                        nc.tensor.matmul(sp[:, k * TG:(k + 1) * TG],
                                         krot[lk][:, c * 128:(c + 1) * 128],
                                         qrot[lh][:], start=True, stop=True)
                    pt = ptp.tile([128, 2 * TG], F32R, tag="pt", name=f"pt{qg}_{lh}_{pi}")
                    w = len(pair) * TG
                    nc.scalar.activation(pt[:, 0:w], sp[:, 0:w], EXP, scale=SCALE)
                    for k, (c, mi) in enumerate(pair):
                        if mi >= 0:
                            if mask_resident:
                                mtile = mt_t[mi][:]
                            else:
                                mst = msp.tile([128, TG], F32R, tag="ms",
                                               name=f"ms{qg}_{lh}_{pi}_{k}")
                                nc.sync.dma_start(mst[:], mtd[mi, :, :])
                                mtile = mst[:]
                            nc.vector.tensor_mul(pt[:, k * TG:(k + 1) * TG],
                                                 pt[:, k * TG:(k + 1) * TG], mtile)
                    pts.append((pt, pair))
                    pending.append(pi)
                    if pi == 1:
                        finish_normalize()   # previous head, hidden under refill
                    if len(pending) > 1:
                        emit_av_pair(pending.pop(0))
                if len(pairs) < 2:
                    finish_normalize()
                for pi in pending:
                    emit_av_pair(pi)
                den = rcp.tile([1, TG], F32R, tag="den", name=f"den{qg}_{lh}")
                nc.scalar.activation(den[:], dd[:], mybir.ActivationFunctionType.Copy,
                                     bias=1e-30)
                norm_pending.append((lh, od, dd, den))
            finish_normalize()
            return at_t

        def emit_wo(qg, at_t):
            for ts in range(4):
                for eb in range(NTG):
                    wps = singp.tile([128, TG], F32, tag="sg", name=f"w{qg}_{ts}_{eb}")
                    for lh in range(4):
                        nc.tensor.matmul(wps[:], at_t[lh][:, ts * 128:(ts + 1) * 128],
                                         wo_t[lh][:, eb * TG:(eb + 1) * TG],
                                         start=(lh == 0), stop=(lh == 3))
                    ob = obp.tile([128, TG], F32, tag="ob", name=f"ob{qg}_{ts}_{eb}")
                    nc.scalar.copy(ob[:], wps[:])
                    nc.sync.dma_start(
                        out[qg * TG + ts * 128:qg * TG + (ts + 1) * 128,
                            eb * TG:(eb + 1) * TG], ob[:])

        if causal_compat:
            # schedule: proj0 attn0 proj1 wo0 attn1 proj2 wo1 attn2 proj3 wo2 attn3 wo3
            qrot = emit_proj(0)
            for i in range(len(mt_t)):
                nc.sync.dma_start(mt_t[i][:], mtd[i, :, :])
            for lh in range(4):
                nc.sync.dma_start(wo_t[lh][:], wog[lh * 128:(lh + 1) * 128, :])
            for tg in range(NTG):
                at_t = emit_attention(tg, qrot)
                if tg + 1 < NTG:
                    qrot = emit_proj(tg + 1)
                emit_wo(tg, at_t)
        else:
            # general masks: project everything (Q^T spilled to DRAM), then attend
            for i in range(len(mt_t)):
                nc.sync.dma_start(mt_t[i][:], mtd[i, :, :])
            for lh in range(4):
                nc.sync.dma_start(wo_t[lh][:], wog[lh * 128:(lh + 1) * 128, :])
            for tg in range(NTG):
                emit_proj(tg)
            for qg in range(NTG):
                qrot = [atp.tile([128, TG], F32R, tag=f"qr{lh}", name=f"ql{qg}_{lh}")
                        for lh in range(4)]
                for lh in range(4):
                    nc.sync.dma_start(qrot[lh][:], qscr[qg, lh, :, :])
                at_t = emit_attention(qg, qrot)
                emit_wo(qg, at_t)

    nc.compile()
    return nc


def _host_prep(x, mask, wq, wk, wv, wo):
    m2dm = np.asarray(mask).reshape(T, T) != 0
    cfg = []
    mask_tiles = []
    mask_key = {}
    for qg in range(NTG):
        qs = slice(qg * TG, (qg + 1) * TG)
        lst = []
        for c in range(T // 128):
            sub = m2dm[qs, c * 128:(c + 1) * 128]
            if not sub.any():
                continue
            if sub.all():
                lst.append((c, -1))
            else:
                tilea = np.ascontiguousarray(sub.T).astype(np.float32)
                key = tilea.tobytes()
                if key not in mask_key:
                    mask_key[key] = len(mask_tiles)
                    mask_tiles.append(tilea)
                lst.append((c, mask_key[key]))
        cfg.append(tuple(lst))
    cfg = tuple(cfg)
    if mask_tiles:
        mt = np.stack(mask_tiles, 0)
    else:
        mt = np.zeros((1, 128, TG), np.float32)

    inv = 1.0 / (10000.0 ** (np.arange(64, dtype=np.float64) / 64))
    ang = np.arange(T, dtype=np.float64)[:, None] * inv[None, :]
    c64 = np.cos(ang).astype(np.float32).T
    s64 = np.sin(ang).astype(np.float32).T
    c2 = np.ascontiguousarray(np.concatenate([c64, c64], 0))
    s2 = np.ascontiguousarray(np.concatenate([s64, s64], 0))
    ones = np.ones((128, 128), np.float32)

    in_maps = []
    for b in range(B):
        xTb = np.ascontiguousarray(x[b].T)
        for g in range(4):
            heads = [2 * g, 2 * g + 1, 2 * g + 8, 2 * g + 9]
            kvh = [2 * g, 2 * g + 1]
            wq_a = np.empty((E, 4 * D), np.float32)
            for lh, h in enumerate(heads):
                wq_a[:, lh * D:lh * D + 64] = wq[:, h * D:(h + 1) * D:2]
                wq_a[:, lh * D + 64:(lh + 1) * D] = wq[:, h * D + 1:(h + 1) * D:2]
            wk_a = np.empty((E, 2 * D), np.float32)
            for lk, h in enumerate(kvh):
                wk_a[:, lk * D:lk * D + 64] = wk[:, h * D:(h + 1) * D:2]
                wk_a[:, lk * D + 64:(lk + 1) * D] = wk[:, h * D + 1:(h + 1) * D:2]
            wv_g = np.ascontiguousarray(wv[:, kvh[0] * D:(kvh[0] + 2) * D])
            wo_g = np.ascontiguousarray(
                np.concatenate([wo[h * D:(h + 1) * D] for h in heads], 0))
            in_maps.append({
                "xT": xTb, "wqa": wq_a, "wka": wk_a, "wvg": wv_g, "wog": wo_g,
                "c2": c2, "s2": s2, "ones": ones, "mt": mt,
            })
    return cfg, in_maps


def kernel(x, mask, wq, wk, wv, wo, _profile=None):
    x = np.ascontiguousarray(np.asarray(x, dtype=np.float32))
    wq = np.asarray(wq, dtype=np.float32)
    wk = np.asarray(wk, dtype=np.float32)
    wv = np.asarray(wv, dtype=np.float32)
    wo = np.asarray(wo, dtype=np.float32)
    cfg, in_maps = _host_prep(x, mask, wq, wk, wv, wo)
    if cfg not in _cache:
        _cache[cfg] = _build_program(cfg)
    nc = _cache[cfg]
    kwargs = dict(_profile) if _profile else {}
    res = run_bass_kernel_spmd(nc, in_maps, core_ids=list(range(8)), **kwargs)
    if _profile is not None and isinstance(_profile, dict):
        _profile["result"] = res
    outp = np.zeros((B, T, E), np.float32)
    for b in range(B):
        for g in range(4):
            outp[b] += res.results[b * 4 + g]["o"]
    return outp
